# revision 1
# baseline (speedup 1.0000x reference)
"""EnhancedGovernanceAttention Trainium2 kernel (8 NeuronCores, SPMD).

Sharding: core c owns heads {2c, 2c+1} for BOTH batches (policy_mask is
per-head and batch-shared, so each policy slice is loaded once per core).
Each core computes its heads' attention and a row-parallel partial of the
Wo projection; the host sums the 8 partials and adds bo.

Math notes (vs the jax reference):
 - softmax max-subtraction is dropped: scores ~ N(0,1) + bias in [0,0.2],
   so exp() cannot overflow in fp32; softmax is shift-invariant.
 - log1p memory bias: softmax(s + log(w)) == (w * exp(s)) / sum(w * exp(s))
   with w = 1 + GS*mw + 1e-8, so w is folded into V rows and into the
   denominator matmul -- no per-score log bias needed.
 - causal mask: only lower-triangle k-tiles are computed; the intra-tile
   diagonal mask is baked into the (bf16) policy bias as -40.
 - scores are computed TRANSPOSED ([k, q]) so the PV matmul directly
   yields attn^T, which is the lhsT the output projection needs.
 - x^T is produced by bf16 hi/lo DMA-transposes + one DVE add (exact to
   ~2^-16 relative), avoiding PE-transpose traffic for x.
 - matmuls run in float32r (~1.8e-4 quantization, 4x faster than fp32).
"""

import numpy as np
import ml_dtypes
from contextlib import ExitStack

import concourse.bass as bass
import concourse.tile as tile
from concourse import bacc, mybir
from concourse.bass_utils import run_bass_kernel_spmd
from concourse.masks import make_identity

B, S, D, H, HD = 2, 2048, 2048, 16, 128
GS = 0.1
ROPE_BASE = 10000.0
NCORES = 8
HPC = H // NCORES          # heads per core = 2
SCALE = float(HD) ** -0.5
DT = D // 128              # 16 d-tiles
ST = S // 128              # 16 s-tiles (also k-tiles)
QB = 512                   # q-block width (phase B)
NQB = S // QB              # 4 q-blocks
SB = 256                   # s-block width (phase A panels)
NSB = S // SB              # 8 s-blocks
MASK_NEG = -40.0
SLAB_K = 2                 # k-tiles per bias slab load

F32 = mybir.dt.float32
F32R = mybir.dt.float32r
BF16 = mybir.dt.bfloat16

_CACHE = {}


def build_nc():
    nc = bacc.Bacc("TRN2", target_bir_lowering=False, debug=False,
                   num_devices=NCORES)

    d_xhi = nc.dram_tensor("xhi", [B, S, D], BF16, kind="ExternalInput").ap()
    d_xlo = nc.dram_tensor("xlo", [B, S, D], BF16, kind="ExternalInput").ap()
    d_wq = nc.dram_tensor("wq", [D, HPC * HD], F32R, kind="ExternalInput").ap()
    d_wk = nc.dram_tensor("wk", [D, HPC * HD], F32R, kind="ExternalInput").ap()
    d_wv = nc.dram_tensor("wv", [D, HPC * HD], F32R, kind="ExternalInput").ap()
    d_wo = nc.dram_tensor("wo", [HPC * HD, D], F32R, kind="ExternalInput").ap()
    d_bias = nc.dram_tensor("biasT", [HPC, S, S], BF16, kind="ExternalInput").ap()
    d_wr = nc.dram_tensor("wr", [B, S], F32R, kind="ExternalInput").ap()
    d_w32 = nc.dram_tensor("w32", [B, S], F32, kind="ExternalInput").ap()
    d_cs = nc.dram_tensor("cs", [128, S], F32, kind="ExternalInput").ap()
    d_y = nc.dram_tensor("y", [B, S, D], F32, kind="ExternalOutput").ap()

    with tile.TileContext(nc) as tc, ExitStack() as ctx:
        consts = ctx.enter_context(tc.tile_pool(name="consts", bufs=1))
        wpool = ctx.enter_context(tc.tile_pool(name="wpool", bufs=1))
        qkv = ctx.enter_context(tc.tile_pool(name="qkv", bufs=1))
        panels = ctx.enter_context(tc.tile_pool(name="panels", bufs=2))
        hilo = ctx.enter_context(tc.tile_pool(name="hilo", bufs=2))
        hilo1 = ctx.enter_context(tc.tile_pool(name="hilo1", bufs=1))
        rope = ctx.enter_context(tc.tile_pool(name="rope", bufs=1))
        slabs = ctx.enter_context(tc.tile_pool(name="slabs", bufs=3))
        expp = ctx.enter_context(tc.tile_pool(name="expp", bufs=4))
        normp = ctx.enter_context(tc.tile_pool(name="normp", bufs=1))
        outp = ctx.enter_context(tc.tile_pool(name="outp", bufs=4))
        psum = ctx.enter_context(tc.tile_pool(name="psum", bufs=3, space="PSUM"))
        psum_pv = ctx.enter_context(tc.tile_pool(name="psum_pv", bufs=3, space="PSUM"))
        psum_l = ctx.enter_context(tc.tile_pool(name="psum_l", bufs=2, space="PSUM"))

        def emit_panel(b, sb_i):
            blk = slice(sb_i * SB, sb_i * SB + SB)
            panel = panels.tile([128, DT, SB], F32R, tag="panel", name="panel")
            thi = hilo.tile([128, DT, SB], BF16, tag="thi", name="thi")
            tlo = hilo1.tile([128, DT, SB], BF16, tag="tlo", name="tlo")
            nc.sync.dma_start_transpose(thi, d_xhi[b, blk, :])
            nc.sync.dma_start_transpose(tlo, d_xlo[b, blk, :])
            half = DT // 2
            nc.vector.tensor_add(
                panel[:, :half, :], thi[:, :half, :], tlo[:, :half, :])
            nc.gpsimd.tensor_add(
                panel[:, half:, :], thi[:, half:, :], tlo[:, half:, :])
            return panel

        panel_cache = {}

        # ---------------- constants (emission order = priority) ----------------
        t_w = {}
        for name, dram in (("wq", d_wq), ("wk", d_wk), ("wv", d_wv)):
            t = wpool.tile([128, DT, HPC * HD], F32R, tag=name, name=name)
            nc.gpsimd.dma_start(t, dram.rearrange("(t p) c -> p t c", p=128))
            t_w[name] = t
        t_cs = consts.tile([128, S], F32, tag="cs")
        nc.gpsimd.dma_start(t_cs, d_cs)
        ident = consts.tile([128, 128], F32, tag="ident")
        make_identity(nc, ident)
        ident_bf = consts.tile([128, 128], BF16, tag="ident_bf")
        make_identity(nc, ident_bf)
        t_w32 = consts.tile([128, B, ST], F32, tag="w32")
        nc.gpsimd.dma_start(t_w32, d_w32.rearrange("b (t p) -> p b t", p=128))
        t_wr = consts.tile([128, B, ST], F32R, tag="wr")
        nc.gpsimd.dma_start(t_wr, d_wr.rearrange("b (t p) -> p b t", p=128))
        t_wo = consts.tile([128, HPC, D], F32R, tag="wo")
        nc.gpsimd.dma_start(t_wo, d_wo.rearrange("(h p) c -> p h c", p=128))

        def emit_c_unit(attnT_ref, b_ref, st, nb):
            ss = slice(st * 128, (st + 1) * 128)
            ns = slice(nb * 512, (nb + 1) * 512)
            ops = psum_pv.tile([128, 512], F32, tag="pv", name="ops")
            for h in range(HPC):
                nc.tensor.matmul(
                    ops, attnT_ref[h][:, ss], t_wo[:, h, ns],
                    start=(h == 0), stop=(h == HPC - 1))
            ob = outp.tile([128, 512], F32, tag="ob")
            nc.vector.tensor_copy(ob, ops)
            nc.scalar.dma_start(d_y[b_ref, ss, ns], ob)

        pending_c = []
        for b in range(B):
            # ============ phase A: x^T panels -> q^T,k^T (RoPE), v ============
            qT = {}
            kT = {}
            vv = {}
            for h in range(HPC):
                qT[h] = qkv.tile([128, S], F32R, tag=f"qT{h}", name=f"qT{h}")
                kT[h] = qkv.tile([128, S], F32R, tag=f"kT{h}", name=f"kT{h}")
                vv[h] = qkv.tile([128, ST, HD], F32R, tag=f"v{h}", name=f"v{h}")

            for sb_i in range(NSB):
                s0 = sb_i * SB
                blk = slice(s0, s0 + SB)
                if (b, sb_i) in panel_cache:
                    panel = panel_cache.pop((b, sb_i))
                else:
                    panel = emit_panel(b, sb_i)

                for h in range(HPC):
                    hc = slice(h * HD, (h + 1) * HD)
                    # --- q^T and k^T with fused RoPE ---
                    for name, dest in (("wq", qT[h]), ("wk", kT[h])):
                        ps = psum.tile([128, SB], F32, tag="mm")
                        for dt in range(DT):
                            nc.tensor.matmul(
                                ps, t_w[name][:, dt, hc], panel[:, dt, :],
                                start=(dt == 0), stop=(dt == DT - 1))
                        t1 = rope.tile([128, SB], F32, tag="t1")
                        t2 = rope.tile([128, SB], F32, tag="t2")
                        # cs rows 0-63 = sinT, rows 64-127 = cosT
                        nc.vector.tensor_mul(
                            t1[0:64, :], ps[0:64, :], t_cs[64:128, blk])
                        nc.vector.tensor_mul(
                            t1[64:128, :], ps[64:128, :], t_cs[64:128, blk])
                        nc.vector.tensor_mul(
                            t2[0:64, :], ps[64:128, :], t_cs[0:64, blk])
                        nc.vector.tensor_mul(
                            t2[64:128, :], ps[0:64, :], t_cs[0:64, blk])
                        # dest = [x1*c - x2*s ; x2*c + x1*s]
                        nc.gpsimd.tensor_sub(
                            dest[0:64, blk], t1[0:64, :], t2[0:64, :])
                        nc.gpsimd.tensor_add(
                            dest[64:128, blk], t1[64:128, :], t2[64:128, :])
                    # --- v (natural layout) via PE transpose of v^T ---
                    ps = psum.tile([128, SB], F32, tag="mm")
                    for dt in range(DT):
                        nc.tensor.matmul(
                            ps, t_w["wv"][:, dt, hc], panel[:, dt, :],
                            start=(dt == 0), stop=(dt == DT - 1))
                    svt = normp.tile([128, SB], F32, tag="svt")
                    nc.scalar.copy(svt, ps)
                    vch = psum.tile([128, SB // 128, 128], F32, tag="mm")
                    for c4 in range(SB // 128):
                        nc.tensor.transpose(
                            vch[:, c4, :], svt[:, c4 * 128:(c4 + 1) * 128], ident)
                    for c4 in range(SB // 128):
                        stile = (s0 // 128) + c4
                        nc.scalar.activation(
                            vv[h][:, stile, :], vch[:, c4, :],
                            mybir.ActivationFunctionType.Copy,
                            scale=t_w32[:, b, stile:stile + 1])
                    # drain carried output units from the previous batch
                    if pending_c:
                        emit_c_unit(*pending_c.pop(0))

            # ====== phases B+C software-pipelined over q-blocks ======
            attnT = qT  # norm(j,h) overwrites qT[h][:, qs] after its last read
            for j in range(NQB):
                qs = slice(j * QB, (j + 1) * QB)
                nk = 4 * (j + 1)          # causal: k-tiles 0..nk-1
                steps_left = HPC * nk
                for h in range(HPC):
                    pv = psum_pv.tile([128, QB], F32, tag="pv")
                    lps = psum_l.tile([1, QB], F32, tag="l", name="lps")
                    for g in range((nk + SLAB_K - 1) // SLAB_K):
                        n = min(SLAB_K, nk - g * SLAB_K)
                        slab = slabs.tile([128, SLAB_K, QB], BF16, tag="slab")
                        k0 = g * SLAB_K * 128
                        slab_eng = nc.gpsimd if g % 2 == 0 else nc.scalar
                        slab_eng.dma_start(
                            slab[:, :n, :],
                            d_bias[h, k0:k0 + n * 128, qs].rearrange(
                                "(m p) q -> p m q", p=128))
                        for ml in range(n):
                            m = g * SLAB_K + ml
                            # columns q < 128*m are fully causal-masked; skip
                            # them, but keep N >= 256 (f32r speed) when useful
                            off = max(0, (m - 4 * j) * 128)
                            qso = slice(j * QB + off, (j + 1) * QB)
                            sc = psum.tile([128, QB], F32, tag="mm")
                            nc.tensor.matmul(
                                sc[:, off:], kT[h][:, m * 128:(m + 1) * 128],
                                qT[h][:, qso],
                                start=True, stop=False)
                            nc.tensor.matmul(
                                sc[:, off:], ident_bf, slab[:, ml, off:],
                                start=False, stop=True, skip_group_check=True)
                            ex = expp.tile([128, QB], F32R, tag="ex")
                            nc.scalar.activation(
                                ex[:, off:], sc[:, off:],
                                mybir.ActivationFunctionType.Exp)
                            nc.tensor.matmul(
                                pv[:, off:], vv[h][:, m, :], ex[:, off:],
                                start=(m == 0), stop=(m == nk - 1),
                                skip_group_check=True)
                            nc.tensor.matmul(
                                lps[:, off:], t_wr[:, b, m:m + 1], ex[:, off:],
                                start=(m == 0), stop=(m == nk - 1),
                                skip_group_check=True)
                            # interleave pending output-projection units
                            if pending_c and (steps_left <= len(pending_c)
                                              or (m + h) % 2 == 0):
                                emit_c_unit(*pending_c.pop(0))
                            steps_left -= 1
                    rl = normp.tile([1, QB], F32, tag="rl")
                    nc.vector.reciprocal(rl, lps)
                    rb = normp.tile([128, QB], F32, tag="rb")
                    nc.gpsimd.partition_broadcast(rb, rl)
                    nc.vector.tensor_mul(attnT[h][:, qs], pv, rb)
                if j < NQB - 1:
                    for c in pending_c:
                        emit_c_unit(*c)
                    pending_c = []
                pending_c = pending_c + [
                    (attnT, b, st, nb) for st in range(4 * j, 4 * j + 4)
                    for nb in range(D // 512)]
            for c in pending_c:
                emit_c_unit(*c)
            pending_c = []

    nc.compile()
    return nc


def _host_prep(x, Wq, Wk, Wv, Wo, policy_mask, memory_weights):
    """Build the per-core input maps."""
    bf = ml_dtypes.bfloat16
    xhi = x.astype(bf)
    xlo = (x.astype(np.float32) - xhi.astype(np.float32)).astype(bf)

    # RoPE tables, transposed: cos2 = [cosT; cosT], sinpm = [-sinT; sinT]
    inv_freq = (1.0 / (ROPE_BASE ** (np.arange(0, HD, 2, dtype=np.float32) / HD)))
    t = np.arange(S, dtype=np.float32)
    freqs = np.outer(t, inv_freq).astype(np.float32)      # [S, 64]
    cosT = np.cos(freqs).T.astype(np.float32)             # [64, S]
    sinT = np.sin(freqs).T.astype(np.float32)
    cs = np.ascontiguousarray(np.concatenate([sinT, cosT], axis=0))

    # memory multiplier w = 1 + GS*mw + 1e-8  (exp(log1p(z)) = 1+z)
    mw = memory_weights.reshape(B, S).astype(np.float64)
    w = (1.0 + GS * mw + 1e-8).astype(np.float32)

    # transposed, causal-masked, pre-scaled policy bias per head (bf16)
    maskT = np.tril(np.full((S, S), MASK_NEG, dtype=np.float32), -1)
    pol = np.asarray(policy_mask, dtype=np.float32)[0]    # [H, S, S]

    in_maps = []
    for c in range(NCORES):
        cols = slice(c * HPC * HD, (c + 1) * HPC * HD)
        bias_c = np.empty((HPC, S, S), dtype=bf)
        for hl in range(HPC):
            hg = c * HPC + hl
            bias_c[hl] = (GS * pol[hg].T + maskT).astype(bf)
        in_maps.append({
            "xhi": xhi, "xlo": xlo,
            "wq": np.ascontiguousarray(Wq[:, cols]),
            "wk": np.ascontiguousarray(Wk[:, cols] * np.float32(SCALE)),
            "wv": np.ascontiguousarray(Wv[:, cols]),
            "wo": np.ascontiguousarray(Wo[cols, :]),
            "biasT": bias_c,
            "wr": w, "w32": w,
            "cs": cs,
        })
    return in_maps


def kernel(x, Wq, Wk, Wv, Wo, bo, policy_mask, memory_weights):
    x = np.asarray(x, dtype=np.float32)
    Wq = np.asarray(Wq, dtype=np.float32)
    Wk = np.asarray(Wk, dtype=np.float32)
    Wv = np.asarray(Wv, dtype=np.float32)
    Wo = np.asarray(Wo, dtype=np.float32)
    bo = np.asarray(bo, dtype=np.float32)

    if "nc" not in _CACHE:
        _CACHE["nc"] = build_nc()
    nc = _CACHE["nc"]

    in_maps = _host_prep(x, Wq, Wk, Wv, Wo, policy_mask, memory_weights)
    res = run_bass_kernel_spmd(nc, in_maps, core_ids=list(range(NCORES)))

    acc = np.zeros((B, S, D), dtype=np.float64)
    for c in range(NCORES):
        acc += res.results[c]["y"].astype(np.float64)
    return (acc + bo.astype(np.float64)).astype(np.float32)



# revision 22
# speedup vs baseline: 1.6808x; 1.6808x over previous
"""EnhancedGovernanceAttention Trainium2 kernel (8 NeuronCores, SPMD).

Sharding: core c owns heads {2c, 2c+1} for BOTH batches. Each core computes
its heads' attention and a row-parallel partial of the Wo projection; the
host sums the 8 partials and adds bo.

Math notes (vs the jax reference):
 - softmax max-subtraction is dropped: scores ~ N(0,1) + bias in [0,0.3],
   so exp() cannot overflow; softmax is shift-invariant.
 - log1p memory bias: log(1 + GS*mw + 1e-8) = log(w) is folded into the
   per-(batch,head) additive bias table, so exp(score+bias) already carries
   w for both the PV numerator and the denominator row-sum.
 - causal mask: only lower-triangle k-tiles are computed; the intra-tile
   diagonal mask is baked into the (fp8) bias as -40.
 - scores are computed TRANSPOSED ([k, q]) so the PV matmul directly
   yields attn^T, which is the lhsT the output projection needs.
 - QKV projections run as fp8 DoubleRow matmuls on a hi/lo split of x and
   64*W (3 cross terms; the 64x pre-scale keeps the lo residuals out of
   fp8's subnormal range; 1/64 is folded into the RoPE tables and the V
   staging copy). Everything else runs bf16.
 - the policy bias is added into the score PSUM with an fp8 DoubleRow
   identity matmul (2 k-tiles per slab, [I;0]/[0;I] selects the slot).
 - softmax denominator: exp tiles are accumulated into U (alternating
   DVE/Pool adds); one ones^T @ U matmul per q-block yields the row sums.
"""

import numpy as np
import ml_dtypes
from contextlib import ExitStack

import concourse.bass as bass
import concourse.tile as tile
from concourse import bacc, mybir
from concourse.bass_utils import run_bass_kernel_spmd
from concourse.masks import make_identity

B, S, D, H, HD = 2, 2048, 2048, 16, 128
GS = 0.1
ROPE_BASE = 10000.0
NCORES = 8
HPC = H // NCORES          # heads per core = 2
SCALE = float(HD) ** -0.5
DT = D // 128              # 16 d-tiles
GT = DT // 2               # 8 d-tile pairs (DoubleRow)
ST = S // 128              # 16 s-tiles (also k-tiles)
QB = 512                   # q-block width (phase B)
NQB = S // QB              # 4 q-blocks
SB = 512                   # s-block width (phase A panels)
NSB = S // SB              # 4 s-blocks
MASK_NEG = -40.0
WSC = 64.0                 # fp8 weight pre-scale

F32 = mybir.dt.float32
F32R = mybir.dt.float32r
BF16 = mybir.dt.bfloat16
FP8 = mybir.dt.float8e4
EXP = mybir.ActivationFunctionType.Exp
CPY = mybir.ActivationFunctionType.Copy
DR = mybir.MatmulPerfMode.DoubleRow

_CACHE = {}


def build_nc():
    nc = bacc.Bacc("TRN2", target_bir_lowering=False, debug=False,
                   num_devices=NCORES)

    d_xhi = nc.dram_tensor("xhi", [B, 128, GT, 2, S], FP8, kind="ExternalInput").ap()
    d_xlo = nc.dram_tensor("xlo", [B, 128, GT, 2, S], FP8, kind="ExternalInput").ap()
    CC = HPC * HD
    d_w = {}
    for nm in ("qhi", "qlo", "khi", "klo", "vhi", "vlo"):
        d_w[nm] = nc.dram_tensor(f"w{nm}", [128, GT, 2, CC], FP8,
                                 kind="ExternalInput").ap()
    d_wo = nc.dram_tensor("wo", [128, HPC, D], BF16, kind="ExternalInput").ap()
    d_id8 = nc.dram_tensor("id8", [128, 3, 128], FP8, kind="ExternalInput").ap()
    d_ones = nc.dram_tensor("ones", [128, 1], F32R, kind="ExternalInput").ap()
    d_bias = nc.dram_tensor("biasT", [B, HPC, S, S], FP8, kind="ExternalInput").ap()
    d_cs = nc.dram_tensor("cs", [128, 2, S], BF16, kind="ExternalInput").ap()
    d_y = nc.dram_tensor("y", [B, S, D], BF16, kind="ExternalOutput").ap()

    with tile.TileContext(nc) as tc, ExitStack() as ctx:
        consts = ctx.enter_context(tc.tile_pool(name="consts", bufs=1))
        wpool = ctx.enter_context(tc.tile_pool(name="wpool", bufs=1))
        qkv = ctx.enter_context(tc.tile_pool(name="qkv", bufs=2))
        panels = ctx.enter_context(tc.tile_pool(name="panels", bufs=3))
        rope = ctx.enter_context(tc.tile_pool(name="rope", bufs=4))
        svtp = ctx.enter_context(tc.tile_pool(name="svtp", bufs=2))
        slabs = ctx.enter_context(tc.tile_pool(name="slabs", bufs=4))
        expp = ctx.enter_context(tc.tile_pool(name="expp", bufs=4))
        upool = ctx.enter_context(tc.tile_pool(name="upool", bufs=2))
        normp = ctx.enter_context(tc.tile_pool(name="normp", bufs=2))
        outp = ctx.enter_context(tc.tile_pool(name="outp", bufs=8))
        psum_mm = ctx.enter_context(tc.tile_pool(name="psum_mm", bufs=3, space="PSUM"))
        psum_pv = ctx.enter_context(tc.tile_pool(name="psum_pv", bufs=2, space="PSUM"))
        psum_c = ctx.enter_context(tc.tile_pool(name="psum_c", bufs=2, space="PSUM"))
        psum_l = ctx.enter_context(tc.tile_pool(name="psum_l", bufs=1, space="PSUM"))

        # ---------------- constants (emission order = queue priority) ----------
        # ACT queue: q/k weights (first two chains); Pool: cs then v weights.
        t_w = {}
        for nm in ("qhi", "khi", "qlo", "klo", "vhi", "vlo"):
            t_w[nm] = wpool.tile([128, GT, 2, CC], FP8, tag=f"w{nm}", name=f"w{nm}")
            nc.scalar.dma_start(t_w[nm], d_w[nm])
        t_cs = consts.tile([128, 2, S], BF16, tag="cs")   # DMA'd on SP below

        ident_bf = consts.tile([128, 128], BF16, tag="ident_bf")
        make_identity(nc, ident_bf)
        # [I, 0, I] in fp8: id8[:, 0:2] = [I;0] (even k-tile), id8[:, 1:3] = [0;I]
        # host-loaded: on-device fp8/f32r init breaks the walrus backend.
        id8 = consts.tile([128, 3, 128], FP8, tag="id8")
        nc.gpsimd.dma_start(id8, d_id8)
        onesR = consts.tile([128, 1], F32R, tag="ones")
        nc.gpsimd.dma_start(onesR, d_ones)
        t_wo = consts.tile([128, HPC, D], BF16, tag="wo")   # DMA deferred

        # ---------------- helpers ------------------------------------------
        panel_cache = {}

        def emit_panel(b, sb_i, split=False):
            blk = slice(sb_i * SB, sb_i * SB + SB)
            phi = panels.tile([128, GT, 2, SB], FP8, tag="phi", name="phi")
            plo = panels.tile([128, GT, 2, SB], FP8, tag="plo", name="plo")
            if split:  # halve the first transfer so the first chain starts early
                nc.sync.dma_start(phi[:, 0:GT // 2], d_xhi[b, :, 0:GT // 2, :, blk])
                nc.sync.dma_start(phi[:, GT // 2:], d_xhi[b, :, GT // 2:, :, blk])
            else:
                nc.sync.dma_start(phi, d_xhi[b, :, :, :, blk])
            nc.sync.dma_start(plo, d_xlo[b, :, :, :, blk])
            if b == 0 and sb_i < NSB:
                # cs chunk for this block, after the panel: RoPE needs it
                # later than the matmul chains need the panel.
                nc.sync.dma_start(t_cs[:, :, blk], d_cs[:, :, blk])
            return phi, plo

        def dr_part(ps, terms, start, stop, gr=None):
            gr = gr if gr is not None else range(GT)
            n = len(terms) * len(gr)
            idx = 0
            for wt, xt, hc in terms:
                for g in gr:
                    nc.tensor.matmul(
                        ps, wt[:, g, :, hc], xt[:, g, :, :],
                        start=(start and idx == 0),
                        stop=(stop and idx == n - 1),
                        perf_mode=DR, skip_group_check=True)
                    idx += 1

        def dr_chain(ps, whi, wlo, phi, plo, hc):
            # sum of 3 fp8 DoubleRow cross terms; hi*hi first so the chain
            # can start before the lo tensors arrive.
            dr_part(ps, [(whi, phi, hc), (whi, plo, hc), (wlo, phi, hc)],
                    True, True)

        ncp = [0]
        pending_y = []

        def flush_y(k=1):
            # y-DMAs are emitted one C-unit late so the (in-order) issuing
            # SEQ never parks on a not-yet-copied ob tile.
            for _ in range(k):
                if pending_y:
                    dst, ob = pending_y.pop(0)
                    if draining[0]:
                        eng = (nc.gpsimd, nc.sync, nc.scalar)[ncp[0] % 3]
                    else:
                        eng = nc.gpsimd if ncp[0] % 2 == 0 else nc.sync
                    eng.dma_start(dst, ob)

        draining = [False]

        def emit_c_unit(attnT_ref, b_ref, st, nb):
            ss = slice(st * 128, (st + 1) * 128)
            ns = slice(nb * 512, (nb + 1) * 512)
            # during the final drain the score pool is idle: borrow its banks
            # to deepen the C-unit pipeline.
            if draining[0] and ncp[0] % 3 == 0:
                ops = psum_mm.tile([128, 512], F32, tag="mm", name="ops")
            elif draining[0] and ncp[0] % 3 == 1:
                ops = psum_pv.tile([128, 512], F32, tag="pv", name="ops")
            else:
                ops = psum_c.tile([128, 512], F32, tag="c", name="ops")
            for h in range(HPC):
                nc.tensor.matmul(
                    ops, attnT_ref[h][:, ss], t_wo[:, h, ns],
                    start=(h == 0), stop=(h == HPC - 1))
            ob = outp.tile([128, 512], BF16, tag="ob")
            ncp[0] += 1
            if draining[0]:
                nc.vector.tensor_copy(ob[:, 0:256], ops[:, 0:256])
                nc.scalar.copy(ob[:, 256:512], ops[:, 256:512])
            elif ncp[0] % 2 == 0:
                nc.scalar.copy(ob, ops)
            else:
                nc.vector.tensor_copy(ob, ops)
            pending_y.append((d_y[b_ref, ss, ns], ob))
            flush_y(1)

        # deferred work queue: (kind, closure) entries — normalization tails
        # and C-units — that fill PE gaps in later m-loops / phase-A blocks.
        # A tail's lps matmul waits on the previous block's exp/U chain, so
        # tails are only popped when `late` (the consumer has caught up);
        # C-units never jump ahead of their own block's tails.
        fill_q = []

        def pop_fill(k=1, late=True):
            for _ in range(k):
                if not fill_q:
                    return
                if fill_q[0][0] == "tail" and not late:
                    return
                fill_q.pop(0)[1]()

        for b in range(B):
            # ============ phase A: x^T panels -> q^T,k^T (RoPE), v ============
            qT = {}
            kT = {}
            vv = {}
            for h in range(HPC):
                qT[h] = qkv.tile([128, S], BF16, tag=f"qT{h}", name=f"qT{h}")
                kT[h] = qkv.tile([128, S], BF16, tag=f"kT{h}", name=f"kT{h}")
                vv[h] = qkv.tile([128, ST, HD], BF16, tag=f"v{h}", name=f"v{h}")

            for sb_i in range(NSB):
                s0 = sb_i * SB
                blk = slice(s0, s0 + SB)
                if (b, sb_i) in panel_cache:
                    phi, plo = panel_cache.pop((b, sb_i))
                else:
                    phi, plo = emit_panel(b, sb_i, split=(b == 0 and sb_i == 0))

                def rope_emit(ps, dest):
                    # cs slot 0 = [cosT;cosT]/64, slot 1 = [-sinT;+sinT]/64
                    t1 = rope.tile([128, SB], F32, tag="t1")
                    t2 = rope.tile([128, SB], F32, tag="t2")
                    nc.vector.tensor_mul(t1, ps, t_cs[:, 0, blk])
                    nc.vector.tensor_mul(
                        t2[0:64, :], ps[64:128, :], t_cs[0:64, 1, blk])
                    nc.vector.tensor_mul(
                        t2[64:128, :], ps[0:64, :], t_cs[64:128, 1, blk])
                    nc.vector.tensor_add(dest[:, blk], t1, t2)

                # q,k chains (both heads) first, then v: the first v chain
                # then starts after wv has streamed in.
                first = b == 0 and sb_i == 0
                if first:
                    # cold start: hi*hi parts of all four q/k chains first
                    # (split by panel half), so the PE works while the lo
                    # tensors are still streaming in.
                    chains = [(pre, h) for h in range(HPC) for pre in ("q", "k")]
                    pss = {}
                    for i, (pre, h) in enumerate(chains):
                        hc = slice(h * HD, (h + 1) * HD)
                        pool, tag = ((psum_mm, "mm") if i < 3 else
                                     (psum_c, "c"))
                        ps = pool.tile([128, SB], F32, tag=tag, name="pss")
                        pss[(pre, h)] = ps
                        dr_part(ps, [(t_w[pre + "hi"], phi, hc)], True, False,
                                gr=range(GT // 2))
                    for pre, h in chains:
                        hc = slice(h * HD, (h + 1) * HD)
                        dr_part(pss[(pre, h)], [(t_w[pre + "hi"], phi, hc)],
                                False, False, gr=range(GT // 2, GT))
                    for pre, h in chains:
                        hc = slice(h * HD, (h + 1) * HD)
                        ps = pss[(pre, h)]
                        dr_part(ps, [(t_w[pre + "hi"], plo, hc),
                                     (t_w[pre + "lo"], phi, hc)], False, True)
                        rope_emit(ps, qT[h] if pre == "q" else kT[h])
                else:
                    for h in range(HPC):
                        hc = slice(h * HD, (h + 1) * HD)
                        for pre, dest in (("q", qT[h]), ("k", kT[h])):
                            ps = psum_mm.tile([128, SB], F32, tag="mm")
                            dr_chain(ps, t_w[pre + "hi"], t_w[pre + "lo"],
                                     phi, plo, hc)
                            rope_emit(ps, dest)
                        pop_fill(1)
                # v chains for both heads first; transposes deferred so the
                # PE never waits on the svt staging copy.
                svts = []
                for h in range(HPC):
                    hc = slice(h * HD, (h + 1) * HD)
                    ps = psum_mm.tile([128, SB], F32, tag="mm")
                    dr_chain(ps, t_w["vhi"], t_w["vlo"], phi, plo, hc)
                    svt = svtp.tile([128, SB], BF16, tag="svt")
                    nc.scalar.activation(svt, ps, CPY, 0.0, 1.0 / WSC)
                    svts.append(svt)
                    pop_fill(1)
                for h in range(HPC):
                    vtr = psum_c.tile([128, SB // 128, 128], BF16, tag="c")
                    for c4 in range(SB // 128):
                        nc.tensor.transpose(
                            vtr[:, c4, :], svts[h][:, c4 * 128:(c4 + 1) * 128],
                            ident_bf)
                    nc.scalar.copy(
                        vv[h][:, sb_i * (SB // 128):(sb_i + 1) * (SB // 128), :],
                        vtr)
                    pop_fill(1)
                if b == 0 and sb_i == 0:
                    nc.gpsimd.dma_start(t_wo, d_wo)   # needed from j=1 on

            # ====== phases B+C software-pipelined over q-blocks ======
            attnT = qT  # norm(j,h) overwrites qT[h][:, qs] after its last read
            for j in range(NQB):
                qs = slice(j * QB, (j + 1) * QB)
                nk = 4 * (j + 1)          # causal: k-tiles 0..nk-1
                for h in range(HPC):
                    pv = psum_pv.tile([128, QB], F32, tag="pv", name="pv")
                    U = upool.tile([128, QB], F32R, tag="U", name="U")
                    offs = [max(0, (m - 4 * j) * 128) for m in range(nk)]

                    def emit_pvu(m, ex):
                        off = offs[m]
                        nc.tensor.matmul(
                            pv[:, off:], vv[h][:, m, :], ex[:, off:],
                            start=(m == 0), stop=(m == nk - 1),
                            skip_group_check=True)
                        if m == 0:
                            nc.vector.tensor_copy(U, ex)
                        else:
                            nc.vector.tensor_add(
                                U[:, off:], U[:, off:], ex[:, off:])

                    # software-skewed m loop: scores/bias/exp of m, then the
                    # PV+U of m-1 (covering the exp latency with C-unit pops).
                    slab = None
                    prev = None
                    for m in range(nk):
                        if m % 2 == 0:
                            slab = slabs.tile([128, 2, QB], FP8, tag="slab")
                            k0 = m * 128
                            nc.sync.dma_start(
                                slab,
                                d_bias[b, h, k0:k0 + 256, qs].rearrange(
                                    "(m p) q -> p m q", p=128))
                        ml = m % 2
                        off = offs[m]
                        qso = slice(j * QB + off, (j + 1) * QB)
                        sc = psum_mm.tile([128, QB], F32, tag="mm")
                        nc.tensor.matmul(
                            sc[:, off:], kT[h][:, m * 128:(m + 1) * 128],
                            qT[h][:, qso],
                            start=True, stop=False)
                        nc.tensor.matmul(
                            sc[:, off:], id8[:, ml:ml + 2, :],
                            slab[:, :, off:],
                            start=False, stop=True, perf_mode=DR,
                            skip_group_check=True)
                        ex = expp.tile([128, QB], BF16, tag="ex")
                        nc.scalar.activation(ex[:, off:], sc[:, off:], EXP)
                        if prev is not None:
                            pop_fill(1, late=(m >= 3))
                            emit_pvu(*prev)
                        prev = (m, ex)
                    pop_fill(1, late=True)
                    emit_pvu(*prev)

                    def make_tail(pv=pv, U=U, h=h, qs=qs, at=attnT):
                        def tail():
                            lrow = psum_l.tile([1, QB], F32, tag="l", name="l")
                            nc.tensor.matmul(lrow, onesR, U)
                            rl = normp.tile([1, QB], F32, tag="rl")
                            nc.vector.reciprocal(rl, lrow)
                            rb = normp.tile([128, QB], F32, tag="rb")
                            nc.gpsimd.partition_broadcast(rb, rl)
                            nc.vector.tensor_mul(at[h][:, qs], pv, rb)
                        return tail

                    fill_q.append(("tail", make_tail()))

                    # prefetch next batch's first panels near the end
                    if b + 1 < B and j == NQB - 1 and h == 0:
                        panel_cache[(b + 1, 0)] = emit_panel(b + 1, 0)

                fill_q.extend(
                    ("c", lambda st=st, nb=nb, at=attnT, bb=b:
                     emit_c_unit(at, bb, st, nb))
                    for st in range(4 * j, 4 * j + 4) for nb in range(D // 512))

        draining[0] = True
        while fill_q:
            fill_q.pop(0)[1]()
        flush_y(len(pending_y))

    nc.compile()
    return nc


def _host_prep(x, Wq, Wk, Wv, Wo, policy_mask, memory_weights):
    """Build the per-core input maps."""
    bf = ml_dtypes.bfloat16
    f8 = ml_dtypes.float8_e4m3

    def hilo_tiles(a):
        # [D, C] (or [D, S]) -> hi/lo fp8 in [128, GT, 2, C] DoubleRow layout
        hi = a.astype(f8)
        lo = (a - hi.astype(np.float32)).astype(f8)
        def tl(t):
            return np.ascontiguousarray(
                t.reshape(GT, 2, 128, -1).transpose(2, 0, 1, 3))
        return tl(hi), tl(lo)

    xhi = np.empty((B, 128, GT, 2, S), f8)
    xlo = np.empty((B, 128, GT, 2, S), f8)
    for b in range(B):
        xt = np.ascontiguousarray(np.asarray(x[b], np.float32).T)  # [D, S]
        xhi[b], xlo[b] = hilo_tiles(xt)

    # RoPE tables (carry the 1/WSC weight descale):
    inv_freq = (1.0 / (ROPE_BASE ** (np.arange(0, HD, 2, dtype=np.float32) / HD)))
    t = np.arange(S, dtype=np.float32)
    freqs = np.outer(t, inv_freq).astype(np.float32)      # [S, 64]
    cosT = np.cos(freqs).T.astype(np.float32) / WSC       # [64, S]
    sinT = np.sin(freqs).T.astype(np.float32) / WSC
    cs = np.empty((128, 2, S), np.float32)
    cs[0:64, 0] = cosT
    cs[64:128, 0] = cosT
    cs[0:64, 1] = -sinT
    cs[64:128, 1] = sinT
    cs = cs.astype(bf)

    # memory multiplier w = 1 + GS*mw + 1e-8  (exp(log1p(z)) = 1+z)
    mw = memory_weights.reshape(B, S).astype(np.float64)
    logw = np.log(1.0 + GS * mw + 1e-8).astype(np.float32)  # [B, S]

    # transposed, causal-masked, pre-scaled policy bias per (batch, head)
    maskT = np.tril(np.full((S, S), MASK_NEG, dtype=np.float32), -1)
    pol = np.asarray(policy_mask, dtype=np.float32)[0]    # [H, S, S]

    id8h = np.zeros((128, 3, 128), np.float32)
    id8h[:, 0, :] = np.eye(128, dtype=np.float32)
    id8h[:, 2, :] = np.eye(128, dtype=np.float32)
    id8h = id8h.astype(f8)
    ones1 = np.ones((128, 1), np.float32)

    in_maps = []
    for c in range(NCORES):
        cols = slice(c * HPC * HD, (c + 1) * HPC * HD)
        bias_c = np.empty((B, HPC, S, S), dtype=f8)
        for hl in range(HPC):
            hg = c * HPC + hl
            polT = GS * pol[hg].T + maskT                 # [S(k), S(q)]
            for b in range(B):
                bias_c[b, hl] = (polT + logw[b][:, None]).astype(f8)
        wo_c = np.ascontiguousarray(
            np.asarray(Wo, np.float32)[cols, :]
            .reshape(HPC, 128, D).transpose(1, 0, 2)).astype(bf)
        m = {"xhi": xhi, "xlo": xlo, "wo": wo_c, "biasT": bias_c, "cs": cs,
             "id8": id8h, "ones": ones1}
        for nm, w, s in (("q", Wq, WSC), ("k", Wk, WSC * SCALE), ("v", Wv, WSC)):
            hi, lo = hilo_tiles(np.asarray(w, np.float32)[:, cols] * np.float32(s))
            m[f"w{nm}hi"] = hi
            m[f"w{nm}lo"] = lo
        in_maps.append(m)
    return in_maps


def kernel(x, Wq, Wk, Wv, Wo, bo, policy_mask, memory_weights):
    x = np.asarray(x, dtype=np.float32)
    Wq = np.asarray(Wq, dtype=np.float32)
    Wk = np.asarray(Wk, dtype=np.float32)
    Wv = np.asarray(Wv, dtype=np.float32)
    Wo = np.asarray(Wo, dtype=np.float32)
    bo = np.asarray(bo, dtype=np.float32)

    if "nc" not in _CACHE:
        _CACHE["nc"] = build_nc()
    nc = _CACHE["nc"]

    in_maps = _host_prep(x, Wq, Wk, Wv, Wo, policy_mask, memory_weights)
    res = run_bass_kernel_spmd(nc, in_maps, core_ids=list(range(NCORES)))

    acc = np.zeros((B, S, D), dtype=np.float64)
    for c in range(NCORES):
        acc += res.results[c]["y"].astype(np.float64)
    return (acc + bo.astype(np.float64)).astype(np.float32)


# revision 34
# speedup vs baseline: 1.7228x; 1.0250x over previous
"""EnhancedGovernanceAttention Trainium2 kernel (8 NeuronCores, SPMD).

Sharding: core c owns heads {2c, 2c+1} for BOTH batches. Each core computes
its heads' attention and a row-parallel partial of the Wo projection; the
host sums the 8 partials and adds bo.

Math notes (vs the jax reference):
 - softmax max-subtraction is dropped: scores ~ N(0,1) + bias in [0,0.3],
   so exp() cannot overflow; softmax is shift-invariant.
 - log1p memory bias: log(1 + GS*mw + 1e-8) = log(w) is folded into the
   per-(batch,head) additive bias table, so exp(score+bias) already carries
   w for both the PV numerator and the denominator row-sum.
 - causal mask: only lower-triangle k-tiles are computed; the intra-tile
   diagonal mask is baked into the (fp8) bias as -40.
 - scores are computed TRANSPOSED ([k, q]) so the PV matmul directly
   yields attn^T, which is the lhsT the output projection needs.
 - QKV projections run as fp8 DoubleRow matmuls on a hi/lo split of x and
   64*W (3 cross terms; the 64x pre-scale keeps the lo residuals out of
   fp8's subnormal range; 1/64 is folded into the RoPE tables and the V
   staging copy). Everything else runs bf16.
 - the policy bias is added into the score PSUM with an fp8 DoubleRow
   identity matmul (2 k-tiles per slab, [I;0]/[0;I] selects the slot).
 - softmax denominator: exp tiles are accumulated into U (alternating
   DVE/Pool adds); one ones^T @ U matmul per q-block yields the row sums.
"""

import numpy as np
import ml_dtypes
from contextlib import ExitStack

import concourse.bass as bass
import concourse.tile as tile
from concourse import bacc, mybir
from concourse.bass_utils import run_bass_kernel_spmd
from concourse.masks import make_identity

B, S, D, H, HD = 2, 2048, 2048, 16, 128
GS = 0.1
ROPE_BASE = 10000.0
NCORES = 8
HPC = H // NCORES          # heads per core = 2
SCALE = float(HD) ** -0.5
DT = D // 128              # 16 d-tiles
GT = DT // 2               # 8 d-tile pairs (DoubleRow)
ST = S // 128              # 16 s-tiles (also k-tiles)
QB = 512                   # q-block width (phase B)
NQB = S // QB              # 4 q-blocks
SB = 512                   # s-block width (phase A panels)
NSB = S // SB              # 4 s-blocks
MASK_NEG = -40.0
WSC = 64.0                 # fp8 weight pre-scale

F32 = mybir.dt.float32
F32R = mybir.dt.float32r
BF16 = mybir.dt.bfloat16
FP8 = mybir.dt.float8e4
EXP = mybir.ActivationFunctionType.Exp
CPY = mybir.ActivationFunctionType.Copy
DR = mybir.MatmulPerfMode.DoubleRow

_CACHE = {}


def build_nc():
    nc = bacc.Bacc("TRN2", target_bir_lowering=False, debug=False,
                   num_devices=NCORES)

    d_xhi = nc.dram_tensor("xhi", [B, 128, GT, 2, S], FP8, kind="ExternalInput").ap()
    d_xlo = nc.dram_tensor("xlo", [B, 128, GT, 2, S], FP8, kind="ExternalInput").ap()
    CC = HPC * HD
    d_w = {}
    for nm in ("qhi", "qlo", "khi", "klo", "vhi", "vlo"):
        d_w[nm] = nc.dram_tensor(f"w{nm}", [128, GT, 2, CC], FP8,
                                 kind="ExternalInput").ap()
    d_wo = nc.dram_tensor("wo", [128, HPC, D], BF16, kind="ExternalInput").ap()
    d_id8 = nc.dram_tensor("id8", [128, 3, 128], FP8, kind="ExternalInput").ap()
    d_ones = nc.dram_tensor("ones", [128, 1], F32R, kind="ExternalInput").ap()
    d_onesrow = nc.dram_tensor("onesrow", [1, 128], F32R, kind="ExternalInput").ap()
    d_bias = nc.dram_tensor("biasT", [B, HPC, S, S], FP8, kind="ExternalInput").ap()
    d_cs = nc.dram_tensor("cs", [128, 2, S], BF16, kind="ExternalInput").ap()
    d_y = nc.dram_tensor("y", [B, S, D], BF16, kind="ExternalOutput").ap()

    with tile.TileContext(nc) as tc, ExitStack() as ctx:
        consts = ctx.enter_context(tc.tile_pool(name="consts", bufs=1))
        wpool = ctx.enter_context(tc.tile_pool(name="wpool", bufs=1))
        qkv = ctx.enter_context(tc.tile_pool(name="qkv", bufs=2))
        panels = ctx.enter_context(tc.tile_pool(name="panels", bufs=3))
        rope = ctx.enter_context(tc.tile_pool(name="rope", bufs=4))
        svtp = ctx.enter_context(tc.tile_pool(name="svtp", bufs=2))
        slabs = ctx.enter_context(tc.tile_pool(name="slabs", bufs=4))
        expp = ctx.enter_context(tc.tile_pool(name="expp", bufs=4))
        upool = ctx.enter_context(tc.tile_pool(name="upool", bufs=2))
        normp = ctx.enter_context(tc.tile_pool(name="normp", bufs=2))
        outp = ctx.enter_context(tc.tile_pool(name="outp", bufs=10))
        psum_mm = ctx.enter_context(tc.tile_pool(name="psum_mm", bufs=3, space="PSUM"))
        psum_pv = ctx.enter_context(tc.tile_pool(name="psum_pv", bufs=2, space="PSUM"))
        psum_c = ctx.enter_context(tc.tile_pool(name="psum_c", bufs=2, space="PSUM"))
        psum_l = ctx.enter_context(tc.tile_pool(name="psum_l", bufs=1, space="PSUM"))

        # ---------------- constants (emission order = queue priority) ----------
        # ACT queue: q/k weights (first two chains); Pool: cs then v weights.
        t_w = {}
        for nm in ("qhi", "khi", "qlo", "klo", "vhi", "vlo"):
            t_w[nm] = wpool.tile([128, GT, 2, CC], FP8, tag=f"w{nm}", name=f"w{nm}")
            nc.scalar.dma_start(t_w[nm], d_w[nm])
        t_cs = consts.tile([128, 2, S], BF16, tag="cs")   # DMA'd on SP below

        ident_bf = consts.tile([128, 128], BF16, tag="ident_bf")
        make_identity(nc, ident_bf)
        # PE warm-up: ~6us of dep-free junk matmuls so the p-state ramp hits
        # peak clock before the first real chain (and the DMA-bound startup
        # window never resets it).
        warm = psum_l.tile([128, 128], F32, tag="l", name="warm")
        for _ in range(50):
            nc.tensor.matmul(warm, ident_bf, ident_bf, start=True, stop=True,
                             skip_group_check=True)
        # [I, 0, I] in fp8: id8[:, 0:2] = [I;0] (even k-tile), id8[:, 1:3] = [0;I]
        # host-loaded: on-device fp8/f32r init breaks the walrus backend.
        # (DMAs deferred past the startup window: needed only in phase B.)
        id8 = consts.tile([128, 3, 128], FP8, tag="id8")
        onesR = consts.tile([128, 1], F32R, tag="ones")
        onesrow = consts.tile([1, 128], F32R, tag="onesrow")
        t_wo = consts.tile([128, HPC, D], BF16, tag="wo")

        # ---------------- helpers ------------------------------------------
        panel_cache = {}

        def emit_panel(b, sb_i, split=False):
            blk = slice(sb_i * SB, sb_i * SB + SB)
            phi = panels.tile([128, GT, 2, SB], FP8, tag="phi", name="phi")
            plo = panels.tile([128, GT, 2, SB], FP8, tag="plo", name="plo")
            if split:  # halve the first transfer so the first chain starts early
                nc.sync.dma_start(phi[:, 0:GT // 2], d_xhi[b, :, 0:GT // 2, :, blk])
                nc.sync.dma_start(phi[:, GT // 2:], d_xhi[b, :, GT // 2:, :, blk])
            else:
                nc.sync.dma_start(phi, d_xhi[b, :, :, :, blk])
            nc.sync.dma_start(plo, d_xlo[b, :, :, :, blk])
            if b == 0 and sb_i < NSB:
                # cs chunk for this block, after the panel: RoPE needs it
                # later than the matmul chains need the panel.
                nc.sync.dma_start(t_cs[:, :, blk], d_cs[:, :, blk])
            return phi, plo

        def dr_part(ps, terms, start, stop, gr=None):
            gr = gr if gr is not None else range(GT)
            n = len(terms) * len(gr)
            idx = 0
            for wt, xt, hc in terms:
                for g in gr:
                    nc.tensor.matmul(
                        ps, wt[:, g, :, hc], xt[:, g, :, :],
                        start=(start and idx == 0),
                        stop=(stop and idx == n - 1),
                        perf_mode=DR, skip_group_check=True)
                    idx += 1

        def dr_chain(ps, whi, wlo, phi, plo, hc):
            # sum of 3 fp8 DoubleRow cross terms; hi*hi first so the chain
            # can start before the lo tensors arrive.
            dr_part(ps, [(whi, phi, hc), (whi, plo, hc), (wlo, phi, hc)],
                    True, True)

        ncp = [0]
        pending_y = []

        def flush_y(k=1):
            # y-DMAs are emitted one C-unit late so the (in-order) issuing
            # SEQ never parks on a not-yet-copied ob tile.
            for _ in range(k):
                if pending_y:
                    dst, ob = pending_y.pop(0)
                    if draining[0]:
                        eng = (nc.gpsimd, nc.sync, nc.scalar)[ncp[0] % 3]
                    else:
                        eng = nc.gpsimd if ncp[0] % 2 == 0 else nc.sync
                    eng.dma_start(dst, ob)

        draining = [False]

        def emit_c_unit(attnT_ref, b_ref, st, nb):
            ss = slice(st * 128, (st + 1) * 128)
            ns = slice(nb * 512, (nb + 1) * 512)
            # during the final drain the score pool is idle: borrow its banks
            # to deepen the C-unit pipeline.
            if draining[0] and ncp[0] % 3 == 0:
                ops = psum_mm.tile([128, 512], F32, tag="mm", name="ops")
            elif draining[0] and ncp[0] % 3 == 1:
                ops = psum_pv.tile([128, 512], F32, tag="pv", name="ops")
            else:
                ops = psum_c.tile([128, 512], F32, tag="c", name="ops")
            for h in range(HPC):
                nc.tensor.matmul(
                    ops, attnT_ref[h][:, ss], t_wo[:, h, ns],
                    start=(h == 0), stop=(h == HPC - 1))
            ob = outp.tile([128, 512], BF16, tag="ob")
            ncp[0] += 1
            if draining[0]:
                nc.vector.tensor_copy(ob[:, 0:256], ops[:, 0:256])
                nc.scalar.copy(ob[:, 256:512], ops[:, 256:512])
            elif ncp[0] % 2 == 0:
                nc.scalar.copy(ob, ops)
            else:
                nc.vector.tensor_copy(ob, ops)
            pending_y.append((d_y[b_ref, ss, ns], ob))
            flush_y(1)

        slab_cache = {}

        def emit_slab(b, h, g, qs):
            slab = slabs.tile([128, 2, QB], FP8, tag="slab")
            k0 = g * 256
            nc.sync.dma_start(
                slab,
                d_bias[b, h, k0:k0 + 256, qs].rearrange(
                    "(m p) q -> p m q", p=128))
            return slab

        # deferred work queue: (kind, closure) entries — normalization tails
        # and C-units — that fill PE gaps in later m-loops / phase-A blocks.
        # A tail's lps matmul waits on the previous block's exp/U chain, so
        # tails are only popped when `late` (the consumer has caught up);
        # C-units never jump ahead of their own block's tails.
        fill_q = []

        def pop_fill(k=1, late=True):
            n = 0
            for _ in range(k):
                if not fill_q:
                    return n
                if fill_q[0][0] == "tail" and not late:
                    return n
                fill_q.pop(0)[1]()
                n += 1
            return n

        for b in range(B):
            # ============ phase A: x^T panels -> q^T,k^T (RoPE), v ============
            qT = {}
            kT = {}
            vv = {}
            for h in range(HPC):
                qT[h] = qkv.tile([128, S], BF16, tag=f"qT{h}", name=f"qT{h}")
                kT[h] = qkv.tile([128, S], BF16, tag=f"kT{h}", name=f"kT{h}")
                vv[h] = qkv.tile([128, ST, HD], BF16, tag=f"v{h}", name=f"v{h}")

            for sb_i in range(NSB):
                s0 = sb_i * SB
                blk = slice(s0, s0 + SB)
                if (b, sb_i) in panel_cache:
                    phi, plo = panel_cache.pop((b, sb_i))
                else:
                    phi, plo = emit_panel(b, sb_i, split=(b == 0 and sb_i == 0))

                def rope_emit(ps, dest):
                    # cs slot 0 = [cosT;cosT]/64, slot 1 = [-sinT;+sinT]/64
                    t1 = rope.tile([128, SB], F32, tag="t1")
                    t2 = rope.tile([128, SB], F32, tag="t2")
                    nc.vector.tensor_mul(t1, ps, t_cs[:, 0, blk])
                    nc.vector.tensor_mul(
                        t2[0:64, :], ps[64:128, :], t_cs[0:64, 1, blk])
                    nc.vector.tensor_mul(
                        t2[64:128, :], ps[0:64, :], t_cs[64:128, 1, blk])
                    nc.vector.tensor_add(dest[:, blk], t1, t2)

                # q,k chains (both heads) first, then v: the first v chain
                # then starts after wv has streamed in.
                first = b == 0 and sb_i == 0
                if first:
                    # cold start: hi*hi parts of all four q/k chains first
                    # (split by panel half), so the PE works while the lo
                    # tensors are still streaming in.
                    chains = [(pre, h) for h in range(HPC) for pre in ("q", "k")]
                    pss = {}
                    for i, (pre, h) in enumerate(chains):
                        hc = slice(h * HD, (h + 1) * HD)
                        pool, tag = ((psum_mm, "mm") if i < 3 else
                                     (psum_c, "c"))
                        ps = pool.tile([128, SB], F32, tag=tag, name="pss")
                        pss[(pre, h)] = ps
                        dr_part(ps, [(t_w[pre + "hi"], phi, hc)], True, False,
                                gr=range(GT // 2))
                    for pre, h in chains:
                        hc = slice(h * HD, (h + 1) * HD)
                        dr_part(pss[(pre, h)], [(t_w[pre + "hi"], phi, hc)],
                                False, False, gr=range(GT // 2, GT))
                    for pre, h in chains:
                        hc = slice(h * HD, (h + 1) * HD)
                        ps = pss[(pre, h)]
                        dr_part(ps, [(t_w[pre + "hi"], plo, hc),
                                     (t_w[pre + "lo"], phi, hc)], False, True)
                        rope_emit(ps, qT[h] if pre == "q" else kT[h])
                else:
                    for h in range(HPC):
                        hc = slice(h * HD, (h + 1) * HD)
                        for pre, dest in (("q", qT[h]), ("k", kT[h])):
                            ps = psum_mm.tile([128, SB], F32, tag="mm")
                            dr_chain(ps, t_w[pre + "hi"], t_w[pre + "lo"],
                                     phi, plo, hc)
                            rope_emit(ps, dest)
                        pop_fill(1)
                # v directly in natural [s, hd] layout: the panel is the
                # stationary (DoubleRow) operand, wv the moving one.
                for h in range(HPC):
                    hc = slice(h * HD, (h + 1) * HD)
                    vps = psum_mm.tile([128, SB // 128, HD], F32, tag="mm",
                                       name="vps")
                    for c4 in range(SB // 128):
                        cs4 = slice(c4 * 128, (c4 + 1) * 128)
                        idx = 0
                        for wt, xt in ((t_w["vhi"], phi), (t_w["vhi"], plo),
                                       (t_w["vlo"], phi)):
                            for g in range(GT):
                                nc.tensor.matmul(
                                    vps[:, c4, :], xt[:, g, :, cs4],
                                    wt[:, g, :, hc],
                                    start=(idx == 0), stop=(idx == 3 * GT - 1),
                                    perf_mode=DR, skip_group_check=True)
                                idx += 1
                    # last block's copies off ACT so the first exp of
                    # phase B isn't queued behind them
                    dstv = vv[h][:, sb_i * (SB // 128):(sb_i + 1) * (SB // 128), :]
                    if sb_i == NSB - 1:
                        nc.vector.tensor_scalar_mul(dstv, vps, 1.0 / WSC)
                    else:
                        nc.scalar.activation(dstv, vps, CPY, 0.0, 1.0 / WSC)
                    pop_fill(2)
                if b == 0 and sb_i == 0:
                    # on ACT, not Pool: any DMA in Pool's stream lands before
                    # make_identity in the scheduled order and the warm-up
                    # fillers' semaphore would wait on its completion.
                    nc.scalar.dma_start(t_wo, d_wo)   # needed from j=1 on
                    nc.scalar.dma_start(id8, d_id8)
                    nc.scalar.dma_start(onesR, d_ones)
                    nc.scalar.dma_start(onesrow, d_onesrow)
                if sb_i == 2:
                    # prefetch the first two bias slabs of this batch's j=0
                    for g in range(2):
                        slab_cache[(b, 0, 0, g)] = emit_slab(
                            b, 0, g, slice(0, QB))

            # ====== phases B+C software-pipelined over q-blocks ======
            attnT = qT  # norm(j,h) overwrites qT[h][:, qs] after its last read
            for j in range(NQB):
                qs = slice(j * QB, (j + 1) * QB)
                nk = 4 * (j + 1)          # causal: k-tiles 0..nk-1
                for h in range(HPC):
                    pv = psum_pv.tile([128, QB], F32, tag="pv", name="pv")
                    U = upool.tile([128, QB], F32R, tag="U", name="U")
                    offs = [max(0, (m - 4 * j) * 128) for m in range(nk)]

                    def emit_pvu(m, ex):
                        off = offs[m]
                        nc.tensor.matmul(
                            pv[:, off:], vv[h][:, m, :], ex[:, off:],
                            start=(m == 0), stop=(m == nk - 1),
                            skip_group_check=True)
                        if m == 0:
                            nc.vector.tensor_copy(U, ex)
                        else:
                            nc.vector.tensor_add(
                                U[:, off:], U[:, off:], ex[:, off:])

                    # software-skewed m loop: scores/bias/exp of m, then the
                    # PV+U of m-1 (covering the exp latency with C-unit pops).
                    # When no deferred work exists, a junk matmul into the
                    # next score tile (fully overwritten by its start=True)
                    # keeps the PE busy and the p-state ramp at peak.
                    slab = None
                    prev = None
                    sc_next = None
                    if j == 0 and h == 0:
                        sc_next = psum_mm.tile([128, QB], F32, tag="mm",
                                               name="sc_next")
                        for _ in range(3):
                            nc.tensor.matmul(
                                sc_next, ident_bf, qT[h][:, 0:QB],
                                start=True, stop=True, skip_group_check=True)
                    for m in range(nk):
                        if m % 2 == 0:
                            slab = slab_cache.pop((b, j, h, m // 2), None)
                            if slab is None:
                                slab = emit_slab(b, h, m // 2, qs)
                        ml = m % 2
                        off = offs[m]
                        qso = slice(j * QB + off, (j + 1) * QB)
                        if sc_next is not None:
                            sc = sc_next
                            sc_next = None
                        else:
                            sc = psum_mm.tile([128, QB], F32, tag="mm")
                        nc.tensor.matmul(
                            sc[:, off:], kT[h][:, m * 128:(m + 1) * 128],
                            qT[h][:, qso],
                            start=True, stop=False)
                        nc.tensor.matmul(
                            sc[:, off:], id8[:, ml:ml + 2, :],
                            slab[:, :, off:],
                            start=False, stop=True, perf_mode=DR,
                            skip_group_check=True)
                        ex = expp.tile([128, QB], BF16, tag="ex")
                        nc.scalar.activation(ex[:, off:], sc[:, off:], EXP)
                        if prev is not None:
                            pop_fill(1, late=(m >= 3))
                            emit_pvu(*prev)
                        prev = (m, ex)
                    pop_fill(2, late=True)
                    emit_pvu(*prev)

                    def make_tail(pv=pv, U=U, h=h, qs=qs, at=attnT):
                        def tail():
                            lrow = psum_l.tile([1, QB], F32, tag="l", name="l")
                            nc.tensor.matmul(lrow, onesR, U)
                            rl = normp.tile([1, QB], F32, tag="rl")
                            nc.vector.reciprocal(rl, lrow)
                            rb = normp.tile([128, QB], F32, tag="rb")
                            nc.gpsimd.partition_broadcast(rb, rl)
                            nc.vector.tensor_mul(at[h][:, qs], pv, rb)
                        return tail

                    fill_q.append(("tail", make_tail()))

                    # prefetch next batch's first panels near the end
                    if b + 1 < B and j == NQB - 1 and h == 0:
                        panel_cache[(b + 1, 0)] = emit_panel(b + 1, 0)

                fill_q.extend(
                    ("c", lambda st=st, nb=nb, at=attnT, bb=b:
                     emit_c_unit(at, bb, st, nb))
                    for st in range(4 * j, 4 * j + 4) for nb in range(D // 512))

        draining[0] = True
        while fill_q:
            fill_q.pop(0)[1]()
        flush_y(len(pending_y))

    nc.compile()
    return nc


def _host_prep(x, Wq, Wk, Wv, Wo, policy_mask, memory_weights):
    """Build the per-core input maps."""
    bf = ml_dtypes.bfloat16
    f8 = ml_dtypes.float8_e4m3

    def hilo_tiles(a):
        # [D, C] (or [D, S]) -> hi/lo fp8 in [128, GT, 2, C] DoubleRow layout
        hi = a.astype(f8)
        lo = (a - hi.astype(np.float32)).astype(f8)
        def tl(t):
            return np.ascontiguousarray(
                t.reshape(GT, 2, 128, -1).transpose(2, 0, 1, 3))
        return tl(hi), tl(lo)

    xhi = np.empty((B, 128, GT, 2, S), f8)
    xlo = np.empty((B, 128, GT, 2, S), f8)
    for b in range(B):
        xt = np.ascontiguousarray(np.asarray(x[b], np.float32).T)  # [D, S]
        xhi[b], xlo[b] = hilo_tiles(xt)

    # RoPE tables (carry the 1/WSC weight descale):
    inv_freq = (1.0 / (ROPE_BASE ** (np.arange(0, HD, 2, dtype=np.float32) / HD)))
    t = np.arange(S, dtype=np.float32)
    freqs = np.outer(t, inv_freq).astype(np.float32)      # [S, 64]
    cosT = np.cos(freqs).T.astype(np.float32) / WSC       # [64, S]
    sinT = np.sin(freqs).T.astype(np.float32) / WSC
    cs = np.empty((128, 2, S), np.float32)
    cs[0:64, 0] = cosT
    cs[64:128, 0] = cosT
    cs[0:64, 1] = -sinT
    cs[64:128, 1] = sinT
    cs = cs.astype(bf)

    # memory multiplier w = 1 + GS*mw + 1e-8  (exp(log1p(z)) = 1+z)
    mw = memory_weights.reshape(B, S).astype(np.float64)
    logw = np.log(1.0 + GS * mw + 1e-8).astype(np.float32)  # [B, S]

    # transposed, causal-masked, pre-scaled policy bias per (batch, head)
    maskT = np.tril(np.full((S, S), MASK_NEG, dtype=np.float32), -1)
    pol = np.asarray(policy_mask, dtype=np.float32)[0]    # [H, S, S]

    id8h = np.zeros((128, 3, 128), np.float32)
    id8h[:, 0, :] = np.eye(128, dtype=np.float32)
    id8h[:, 2, :] = np.eye(128, dtype=np.float32)
    id8h = id8h.astype(f8)
    ones1 = np.ones((128, 1), np.float32)

    in_maps = []
    for c in range(NCORES):
        cols = slice(c * HPC * HD, (c + 1) * HPC * HD)
        bias_c = np.empty((B, HPC, S, S), dtype=f8)
        for hl in range(HPC):
            hg = c * HPC + hl
            polT = GS * pol[hg].T + maskT                 # [S(k), S(q)]
            for b in range(B):
                bias_c[b, hl] = (polT + logw[b][:, None]).astype(f8)
        wo_c = np.ascontiguousarray(
            np.asarray(Wo, np.float32)[cols, :]
            .reshape(HPC, 128, D).transpose(1, 0, 2)).astype(bf)
        m = {"xhi": xhi, "xlo": xlo, "wo": wo_c, "biasT": bias_c, "cs": cs,
             "id8": id8h, "ones": ones1, "onesrow": ones1.reshape(1, 128)}
        for nm, w, s in (("q", Wq, WSC), ("k", Wk, WSC * SCALE), ("v", Wv, WSC)):
            hi, lo = hilo_tiles(np.asarray(w, np.float32)[:, cols] * np.float32(s))
            m[f"w{nm}hi"] = hi
            m[f"w{nm}lo"] = lo
        in_maps.append(m)
    return in_maps


def kernel(x, Wq, Wk, Wv, Wo, bo, policy_mask, memory_weights):
    x = np.asarray(x, dtype=np.float32)
    Wq = np.asarray(Wq, dtype=np.float32)
    Wk = np.asarray(Wk, dtype=np.float32)
    Wv = np.asarray(Wv, dtype=np.float32)
    Wo = np.asarray(Wo, dtype=np.float32)
    bo = np.asarray(bo, dtype=np.float32)

    if "nc" not in _CACHE:
        _CACHE["nc"] = build_nc()
    nc = _CACHE["nc"]

    in_maps = _host_prep(x, Wq, Wk, Wv, Wo, policy_mask, memory_weights)
    res = run_bass_kernel_spmd(nc, in_maps, core_ids=list(range(NCORES)))

    acc = np.zeros((B, S, D), dtype=np.float64)
    for c in range(NCORES):
        acc += res.results[c]["y"].astype(np.float64)
    return (acc + bo.astype(np.float64)).astype(np.float32)


# revision 37
# speedup vs baseline: 1.7861x; 1.0368x over previous
"""EnhancedGovernanceAttention Trainium2 kernel (8 NeuronCores, SPMD).

Sharding: core c owns heads {2c, 2c+1} for BOTH batches. Each core computes
its heads' attention and a row-parallel partial of the Wo projection; the
host sums the 8 partials and adds bo.

Math notes (vs the jax reference):
 - softmax max-subtraction is dropped: scores ~ N(0,1) + bias in [0,0.3],
   so exp() cannot overflow; softmax is shift-invariant.
 - log1p memory bias: log(1 + GS*mw + 1e-8) = log(w) is folded into the
   per-(batch,head) additive bias table, so exp(score+bias) already carries
   w for both the PV numerator and the denominator row-sum.
 - causal mask: only lower-triangle k-tiles are computed; the intra-tile
   diagonal mask is baked into the (fp8) bias as -40.
 - scores are computed TRANSPOSED ([k, q]) so the PV matmul directly
   yields attn^T, which is the lhsT the output projection needs.
 - QKV projections run as fp8 DoubleRow matmuls on a hi/lo split of x and
   64*W (3 cross terms; the 64x pre-scale keeps the lo residuals out of
   fp8's subnormal range; 1/64 is folded into the RoPE tables and the V
   staging copy). Everything else runs bf16.
 - the policy bias is added into the score PSUM with an fp8 DoubleRow
   identity matmul (2 k-tiles per slab, [I;0]/[0;I] selects the slot).
 - softmax denominator: exp tiles are accumulated into U (alternating
   DVE/Pool adds); one ones^T @ U matmul per q-block yields the row sums.
"""

import numpy as np
import ml_dtypes
from contextlib import ExitStack

import concourse.bass as bass
import concourse.tile as tile
from concourse import bacc, mybir
from concourse.bass_utils import run_bass_kernel_spmd
from concourse.masks import make_identity

B, S, D, H, HD = 2, 2048, 2048, 16, 128
GS = 0.1
ROPE_BASE = 10000.0
NCORES = 8
HPC = H // NCORES          # heads per core = 2
SCALE = float(HD) ** -0.5
DT = D // 128              # 16 d-tiles
GT = DT // 2               # 8 d-tile pairs (DoubleRow)
ST = S // 128              # 16 s-tiles (also k-tiles)
QB = 512                   # q-block width (phase B)
NQB = S // QB              # 4 q-blocks
SB = 512                   # s-block width (phase A panels)
NSB = S // SB              # 4 s-blocks
MASK_NEG = -40.0
WSC = 64.0                 # fp8 weight pre-scale

F32 = mybir.dt.float32
F32R = mybir.dt.float32r
BF16 = mybir.dt.bfloat16
FP8 = mybir.dt.float8e4
EXP = mybir.ActivationFunctionType.Exp
CPY = mybir.ActivationFunctionType.Copy
DR = mybir.MatmulPerfMode.DoubleRow

_CACHE = {}


def build_nc():
    nc = bacc.Bacc("TRN2", target_bir_lowering=False, debug=False,
                   num_devices=NCORES)

    d_xhi = nc.dram_tensor("xhi", [B, 128, GT, 2, S], FP8, kind="ExternalInput").ap()
    d_xlo = nc.dram_tensor("xlo", [B, 128, GT, 2, S], FP8, kind="ExternalInput").ap()
    CC = HPC * HD
    d_w = {}
    for nm in ("qhi", "qlo", "khi", "klo", "vhi", "vlo"):
        d_w[nm] = nc.dram_tensor(f"w{nm}", [128, GT, 2, CC], FP8,
                                 kind="ExternalInput").ap()
    d_wo = nc.dram_tensor("wo", [128, HPC, D], BF16, kind="ExternalInput").ap()
    d_id8 = nc.dram_tensor("id8", [128, 3, 128], FP8, kind="ExternalInput").ap()
    d_ones = nc.dram_tensor("ones", [128, 1], BF16, kind="ExternalInput").ap()
    d_onesrow = nc.dram_tensor("onesrow", [1, 128], F32R, kind="ExternalInput").ap()
    d_bias = nc.dram_tensor("biasT", [B, HPC, S, S], FP8, kind="ExternalInput").ap()
    d_cs = nc.dram_tensor("cs", [128, 2, S], BF16, kind="ExternalInput").ap()
    d_y = nc.dram_tensor("y", [B, S, D], BF16, kind="ExternalOutput").ap()

    with tile.TileContext(nc) as tc, ExitStack() as ctx:
        consts = ctx.enter_context(tc.tile_pool(name="consts", bufs=1))
        wpool = ctx.enter_context(tc.tile_pool(name="wpool", bufs=1))
        qkv = ctx.enter_context(tc.tile_pool(name="qkv", bufs=2))
        panels = ctx.enter_context(tc.tile_pool(name="panels", bufs=3))
        rope = ctx.enter_context(tc.tile_pool(name="rope", bufs=4))
        svtp = ctx.enter_context(tc.tile_pool(name="svtp", bufs=2))
        slabs = ctx.enter_context(tc.tile_pool(name="slabs", bufs=4))
        expp = ctx.enter_context(tc.tile_pool(name="expp", bufs=4))
        upool = ctx.enter_context(tc.tile_pool(name="upool", bufs=2))
        normp = ctx.enter_context(tc.tile_pool(name="normp", bufs=2))
        outp = ctx.enter_context(tc.tile_pool(name="outp", bufs=10))
        psum_mm = ctx.enter_context(tc.tile_pool(name="psum_mm", bufs=3, space="PSUM"))
        psum_pv = ctx.enter_context(tc.tile_pool(name="psum_pv", bufs=2, space="PSUM"))
        psum_c = ctx.enter_context(tc.tile_pool(name="psum_c", bufs=2, space="PSUM"))
        psum_l = ctx.enter_context(tc.tile_pool(name="psum_l", bufs=1, space="PSUM"))

        # ---------------- constants (emission order = queue priority) ----------
        # ACT queue: q/k weights (first two chains); Pool: cs then v weights.
        t_w = {}
        for nm in ("qhi", "khi", "qlo", "klo", "vhi", "vlo"):
            t_w[nm] = wpool.tile([128, GT, 2, CC], FP8, tag=f"w{nm}", name=f"w{nm}")
            nc.scalar.dma_start(t_w[nm], d_w[nm])
        t_cs = consts.tile([128, 2, S], BF16, tag="cs")   # DMA'd on SP below

        ident_bf = consts.tile([128, 128], BF16, tag="ident_bf")
        make_identity(nc, ident_bf)
        # PE warm-up: ~6us of dep-free junk matmuls so the p-state ramp hits
        # peak clock before the first real chain (and the DMA-bound startup
        # window never resets it).
        warm = psum_l.tile([128, 128], F32, tag="l", name="warm")
        for _ in range(50):
            nc.tensor.matmul(warm, ident_bf, ident_bf, start=True, stop=True,
                             skip_group_check=True)
        # [I, 0, I] in fp8: id8[:, 0:2] = [I;0] (even k-tile), id8[:, 1:3] = [0;I]
        # host-loaded: on-device fp8/f32r init breaks the walrus backend.
        # (DMAs deferred past the startup window: needed only in phase B.)
        id8 = consts.tile([128, 3, 128], FP8, tag="id8")
        onesR = consts.tile([128, 1], BF16, tag="ones")
        onesrow = consts.tile([1, 128], F32R, tag="onesrow")
        t_wo = consts.tile([128, HPC, D], BF16, tag="wo")

        # ---------------- helpers ------------------------------------------
        panel_cache = {}

        def emit_panel(b, sb_i, split=False):
            blk = slice(sb_i * SB, sb_i * SB + SB)
            phi = panels.tile([128, GT, 2, SB], FP8, tag="phi", name="phi")
            plo = panels.tile([128, GT, 2, SB], FP8, tag="plo", name="plo")
            if split:  # halve the first transfer so the first chain starts early
                nc.sync.dma_start(phi[:, 0:GT // 2], d_xhi[b, :, 0:GT // 2, :, blk])
                nc.sync.dma_start(phi[:, GT // 2:], d_xhi[b, :, GT // 2:, :, blk])
            else:
                nc.sync.dma_start(phi, d_xhi[b, :, :, :, blk])
            nc.sync.dma_start(plo, d_xlo[b, :, :, :, blk])
            if b == 0 and sb_i < NSB:
                # cs chunk for this block, after the panel: RoPE needs it
                # later than the matmul chains need the panel.
                nc.sync.dma_start(t_cs[:, :, blk], d_cs[:, :, blk])
            return phi, plo

        def dr_part(ps, terms, start, stop, gr=None):
            gr = gr if gr is not None else range(GT)
            n = len(terms) * len(gr)
            idx = 0
            for wt, xt, hc in terms:
                for g in gr:
                    nc.tensor.matmul(
                        ps, wt[:, g, :, hc], xt[:, g, :, :],
                        start=(start and idx == 0),
                        stop=(stop and idx == n - 1),
                        perf_mode=DR, skip_group_check=True)
                    idx += 1

        def dr_chain(ps, whi, wlo, phi, plo, hc):
            # sum of 3 fp8 DoubleRow cross terms; hi*hi first so the chain
            # can start before the lo tensors arrive.
            dr_part(ps, [(whi, phi, hc), (whi, plo, hc), (wlo, phi, hc)],
                    True, True)

        ncp = [0]
        pending_y = []

        def flush_y(k=1):
            # y-DMAs are emitted one C-unit late so the (in-order) issuing
            # SEQ never parks on a not-yet-copied ob tile.
            for _ in range(k):
                if pending_y:
                    dst, ob = pending_y.pop(0)
                    if draining[0]:
                        eng = (nc.gpsimd, nc.sync, nc.scalar)[ncp[0] % 3]
                    else:
                        eng = nc.gpsimd if ncp[0] % 2 == 0 else nc.sync
                    eng.dma_start(dst, ob)

        draining = [False]

        def emit_c_unit(attnT_ref, b_ref, st, nb):
            ss = slice(st * 128, (st + 1) * 128)
            ns = slice(nb * 512, (nb + 1) * 512)
            # during the final drain the score pool is idle: borrow its banks
            # to deepen the C-unit pipeline.
            if draining[0] and ncp[0] % 3 == 0:
                ops = psum_mm.tile([128, 512], F32, tag="mm", name="ops")
            elif draining[0] and ncp[0] % 3 == 1:
                ops = psum_pv.tile([128, 512], F32, tag="pv", name="ops")
            else:
                ops = psum_c.tile([128, 512], F32, tag="c", name="ops")
            for h in range(HPC):
                nc.tensor.matmul(
                    ops, attnT_ref[h][:, ss], t_wo[:, h, ns],
                    start=(h == 0), stop=(h == HPC - 1))
            ob = outp.tile([128, 512], BF16, tag="ob")
            ncp[0] += 1
            if draining[0]:
                nc.vector.tensor_copy(ob[:, 0:256], ops[:, 0:256])
                nc.scalar.copy(ob[:, 256:512], ops[:, 256:512])
            elif ncp[0] % 2 == 0:
                nc.scalar.copy(ob, ops)
            else:
                nc.vector.tensor_copy(ob, ops)
            pending_y.append((d_y[b_ref, ss, ns], ob))
            flush_y(1)

        slab_cache = {}

        def emit_slab(b, h, g, qs):
            slab = slabs.tile([128, 2, QB], FP8, tag="slab")
            k0 = g * 256
            nc.sync.dma_start(
                slab,
                d_bias[b, h, k0:k0 + 256, qs].rearrange(
                    "(m p) q -> p m q", p=128))
            return slab

        # deferred work queue: (kind, closure) entries — normalization tails
        # and C-units — that fill PE gaps in later m-loops / phase-A blocks.
        # A tail's lps matmul waits on the previous block's exp/U chain, so
        # tails are only popped when `late` (the consumer has caught up);
        # C-units never jump ahead of their own block's tails.
        fill_q = []

        def pop_fill(k=1, late=True):
            n = 0
            for _ in range(k):
                if not fill_q:
                    return n
                if fill_q[0][0] == "tail" and not late:
                    return n
                fill_q.pop(0)[1]()
                n += 1
            return n

        for b in range(B):
            # ============ phase A: x^T panels -> q^T,k^T (RoPE), v ============
            qT = {}
            kT = {}
            vv = {}
            for h in range(HPC):
                qT[h] = qkv.tile([128, S], BF16, tag=f"qT{h}", name=f"qT{h}")
                kT[h] = qkv.tile([128, S], BF16, tag=f"kT{h}", name=f"kT{h}")
                vv[h] = qkv.tile([128, ST, HD], BF16, tag=f"v{h}", name=f"v{h}")

            for sb_i in range(NSB):
                s0 = sb_i * SB
                blk = slice(s0, s0 + SB)
                if (b, sb_i) in panel_cache:
                    phi, plo = panel_cache.pop((b, sb_i))
                else:
                    phi, plo = emit_panel(b, sb_i, split=(b == 0 and sb_i == 0))

                def rope_emit(ps, dest):
                    # cs slot 0 = [cosT;cosT]/64, slot 1 = [-sinT;+sinT]/64
                    t1 = rope.tile([128, SB], F32, tag="t1")
                    t2 = rope.tile([128, SB], F32, tag="t2")
                    nc.vector.tensor_mul(t1, ps, t_cs[:, 0, blk])
                    nc.vector.tensor_mul(
                        t2[0:64, :], ps[64:128, :], t_cs[0:64, 1, blk])
                    nc.vector.tensor_mul(
                        t2[64:128, :], ps[0:64, :], t_cs[64:128, 1, blk])
                    nc.vector.tensor_add(dest[:, blk], t1, t2)

                # q,k chains (both heads) first, then v: the first v chain
                # then starts after wv has streamed in.
                first = b == 0 and sb_i == 0
                if first:
                    # cold start: hi*hi parts of all four q/k chains first
                    # (split by panel half), so the PE works while the lo
                    # tensors are still streaming in.
                    chains = [(pre, h) for h in range(HPC) for pre in ("q", "k")]
                    pss = {}
                    for i, (pre, h) in enumerate(chains):
                        hc = slice(h * HD, (h + 1) * HD)
                        pool, tag = ((psum_mm, "mm") if i < 3 else
                                     (psum_c, "c"))
                        ps = pool.tile([128, SB], F32, tag=tag, name="pss")
                        pss[(pre, h)] = ps
                        dr_part(ps, [(t_w[pre + "hi"], phi, hc)], True, False,
                                gr=range(GT // 2))
                    for pre, h in chains:
                        hc = slice(h * HD, (h + 1) * HD)
                        dr_part(pss[(pre, h)], [(t_w[pre + "hi"], phi, hc)],
                                False, False, gr=range(GT // 2, GT))
                    for pre, h in chains:
                        hc = slice(h * HD, (h + 1) * HD)
                        ps = pss[(pre, h)]
                        dr_part(ps, [(t_w[pre + "hi"], plo, hc),
                                     (t_w[pre + "lo"], phi, hc)], False, True)
                        rope_emit(ps, qT[h] if pre == "q" else kT[h])
                else:
                    for h in range(HPC):
                        hc = slice(h * HD, (h + 1) * HD)
                        for pre, dest in (("q", qT[h]), ("k", kT[h])):
                            ps = psum_mm.tile([128, SB], F32, tag="mm")
                            dr_chain(ps, t_w[pre + "hi"], t_w[pre + "lo"],
                                     phi, plo, hc)
                            rope_emit(ps, dest)
                        pop_fill(1)
                # v directly in natural [s, hd] layout: the panel is the
                # stationary (DoubleRow) operand, wv the moving one.
                for h in range(HPC):
                    hc = slice(h * HD, (h + 1) * HD)
                    vps = psum_mm.tile([128, SB // 128, HD], F32, tag="mm",
                                       name="vps")
                    for c4 in range(SB // 128):
                        cs4 = slice(c4 * 128, (c4 + 1) * 128)
                        idx = 0
                        for wt, xt in ((t_w["vhi"], phi), (t_w["vhi"], plo),
                                       (t_w["vlo"], phi)):
                            for g in range(GT):
                                nc.tensor.matmul(
                                    vps[:, c4, :], xt[:, g, :, cs4],
                                    wt[:, g, :, hc],
                                    start=(idx == 0), stop=(idx == 3 * GT - 1),
                                    perf_mode=DR, skip_group_check=True)
                                idx += 1
                    # last block's copies off ACT so the first exp of
                    # phase B isn't queued behind them
                    dstv = vv[h][:, sb_i * (SB // 128):(sb_i + 1) * (SB // 128), :]
                    if sb_i == NSB - 1:
                        nc.vector.tensor_scalar_mul(dstv, vps, 1.0 / WSC)
                    else:
                        nc.scalar.activation(dstv, vps, CPY, 0.0, 1.0 / WSC)
                    pop_fill(2)
                if b == 0 and sb_i == 0:
                    # on ACT, not Pool: any DMA in Pool's stream lands before
                    # make_identity in the scheduled order and the warm-up
                    # fillers' semaphore would wait on its completion.
                    nc.scalar.dma_start(t_wo, d_wo)   # needed from j=1 on
                    nc.scalar.dma_start(id8, d_id8)
                    nc.scalar.dma_start(onesR, d_ones)
                    nc.scalar.dma_start(onesrow, d_onesrow)
                if sb_i == 2:
                    # prefetch the first two bias slabs of this batch's j=0
                    for g in range(2):
                        slab_cache[(b, 0, 0, g)] = emit_slab(
                            b, 0, g, slice(0, QB))

            # ====== phases B+C software-pipelined over q-blocks ======
            attnT = qT  # norm(j,h) overwrites qT[h][:, qs] after its last read
            for j in range(NQB):
                qs = slice(j * QB, (j + 1) * QB)
                nk = 4 * (j + 1)          # causal: k-tiles 0..nk-1
                for h in range(HPC):
                    pv = psum_pv.tile([128, QB], F32, tag="pv", name="pv")
                    U = upool.tile([128, QB], BF16, tag="U", name="U")
                    offs = [max(0, (m - 4 * j) * 128) for m in range(nk)]

                    def emit_pvu(m, ex):
                        off = offs[m]
                        nc.tensor.matmul(
                            pv[:, off:], vv[h][:, m, :], ex[:, off:],
                            start=(m == 0), stop=(m == nk - 1),
                            skip_group_check=True)
                        if m == 0:
                            nc.vector.tensor_copy(U, ex)
                        else:
                            nc.vector.tensor_add(
                                U[:, off:], U[:, off:], ex[:, off:])

                    # software-skewed m loop: scores/bias/exp of m, then the
                    # PV+U of m-1 (covering the exp latency with C-unit pops).
                    # When no deferred work exists, a junk matmul into the
                    # next score tile (fully overwritten by its start=True)
                    # keeps the PE busy and the p-state ramp at peak.
                    slab = None
                    prev = None
                    sc_next = None
                    if j == 0 and h == 0:
                        sc_next = psum_mm.tile([128, QB], F32, tag="mm",
                                               name="sc_next")
                        for _ in range(3):
                            nc.tensor.matmul(
                                sc_next, ident_bf, qT[h][:, 0:QB],
                                start=True, stop=True, skip_group_check=True)
                    for m in range(nk):
                        if m % 2 == 0:
                            slab = slab_cache.pop((b, j, h, m // 2), None)
                            if slab is None:
                                slab = emit_slab(b, h, m // 2, qs)
                        ml = m % 2
                        off = offs[m]
                        qso = slice(j * QB + off, (j + 1) * QB)
                        if sc_next is not None:
                            sc = sc_next
                            sc_next = None
                        else:
                            sc = psum_mm.tile([128, QB], F32, tag="mm")
                        nc.tensor.matmul(
                            sc[:, off:], kT[h][:, m * 128:(m + 1) * 128],
                            qT[h][:, qso],
                            start=True, stop=False)
                        nc.tensor.matmul(
                            sc[:, off:], id8[:, ml:ml + 2, :],
                            slab[:, :, off:],
                            start=False, stop=True, perf_mode=DR,
                            skip_group_check=True)
                        ex = expp.tile([128, QB], BF16, tag="ex")
                        nc.scalar.activation(ex[:, off:], sc[:, off:], EXP)
                        if prev is not None:
                            pop_fill(1, late=(m >= 3))
                            emit_pvu(*prev)
                        prev = (m, ex)
                    pop_fill(2, late=True)
                    emit_pvu(*prev)

                    def make_tail(pv=pv, U=U, h=h, qs=qs, at=attnT):
                        def tail():
                            lrow = psum_l.tile([1, QB], F32, tag="l", name="l")
                            nc.tensor.matmul(lrow, onesR, U)
                            rl = normp.tile([1, QB], F32, tag="rl")
                            nc.vector.reciprocal(rl, lrow)
                            rb = normp.tile([128, QB], F32, tag="rb")
                            nc.gpsimd.partition_broadcast(rb, rl)
                            nc.vector.tensor_mul(at[h][:, qs], pv, rb)
                        return tail

                    fill_q.append(("tail", make_tail()))

                    # prefetch next batch's first panels near the end
                    if b + 1 < B and j == NQB - 1 and h == 0:
                        panel_cache[(b + 1, 0)] = emit_panel(b + 1, 0)

                fill_q.extend(
                    ("c", lambda st=st, nb=nb, at=attnT, bb=b:
                     emit_c_unit(at, bb, st, nb))
                    for st in range(4 * j, 4 * j + 4) for nb in range(D // 512))

        draining[0] = True
        while fill_q:
            fill_q.pop(0)[1]()
        flush_y(len(pending_y))

    nc.compile()
    return nc


def _host_prep(x, Wq, Wk, Wv, Wo, policy_mask, memory_weights):
    """Build the per-core input maps."""
    bf = ml_dtypes.bfloat16
    f8 = ml_dtypes.float8_e4m3

    def hilo_tiles(a):
        # [D, C] (or [D, S]) -> hi/lo fp8 in [128, GT, 2, C] DoubleRow layout
        hi = a.astype(f8)
        lo = (a - hi.astype(np.float32)).astype(f8)
        def tl(t):
            return np.ascontiguousarray(
                t.reshape(GT, 2, 128, -1).transpose(2, 0, 1, 3))
        return tl(hi), tl(lo)

    xhi = np.empty((B, 128, GT, 2, S), f8)
    xlo = np.empty((B, 128, GT, 2, S), f8)
    for b in range(B):
        xt = np.ascontiguousarray(np.asarray(x[b], np.float32).T)  # [D, S]
        xhi[b], xlo[b] = hilo_tiles(xt)

    # RoPE tables (carry the 1/WSC weight descale):
    inv_freq = (1.0 / (ROPE_BASE ** (np.arange(0, HD, 2, dtype=np.float32) / HD)))
    t = np.arange(S, dtype=np.float32)
    freqs = np.outer(t, inv_freq).astype(np.float32)      # [S, 64]
    cosT = np.cos(freqs).T.astype(np.float32) / WSC       # [64, S]
    sinT = np.sin(freqs).T.astype(np.float32) / WSC
    cs = np.empty((128, 2, S), np.float32)
    cs[0:64, 0] = cosT
    cs[64:128, 0] = cosT
    cs[0:64, 1] = -sinT
    cs[64:128, 1] = sinT
    cs = cs.astype(bf)

    # memory multiplier w = 1 + GS*mw + 1e-8  (exp(log1p(z)) = 1+z)
    mw = memory_weights.reshape(B, S).astype(np.float64)
    logw = np.log(1.0 + GS * mw + 1e-8).astype(np.float32)  # [B, S]

    # transposed, causal-masked, pre-scaled policy bias per (batch, head)
    maskT = np.tril(np.full((S, S), MASK_NEG, dtype=np.float32), -1)
    pol = np.asarray(policy_mask, dtype=np.float32)[0]    # [H, S, S]

    id8h = np.zeros((128, 3, 128), np.float32)
    id8h[:, 0, :] = np.eye(128, dtype=np.float32)
    id8h[:, 2, :] = np.eye(128, dtype=np.float32)
    id8h = id8h.astype(f8)
    ones1 = np.ones((128, 1), np.float32)

    in_maps = []
    for c in range(NCORES):
        cols = slice(c * HPC * HD, (c + 1) * HPC * HD)
        bias_c = np.empty((B, HPC, S, S), dtype=f8)
        for hl in range(HPC):
            hg = c * HPC + hl
            polT = GS * pol[hg].T + maskT                 # [S(k), S(q)]
            for b in range(B):
                bias_c[b, hl] = (polT + logw[b][:, None]).astype(f8)
        wo_c = np.ascontiguousarray(
            np.asarray(Wo, np.float32)[cols, :]
            .reshape(HPC, 128, D).transpose(1, 0, 2)).astype(bf)
        m = {"xhi": xhi, "xlo": xlo, "wo": wo_c, "biasT": bias_c, "cs": cs,
             "id8": id8h, "ones": ones1.astype(bf),
             "onesrow": ones1.reshape(1, 128)}
        for nm, w, s in (("q", Wq, WSC), ("k", Wk, WSC * SCALE), ("v", Wv, WSC)):
            hi, lo = hilo_tiles(np.asarray(w, np.float32)[:, cols] * np.float32(s))
            m[f"w{nm}hi"] = hi
            m[f"w{nm}lo"] = lo
        in_maps.append(m)
    return in_maps


def kernel(x, Wq, Wk, Wv, Wo, bo, policy_mask, memory_weights):
    x = np.asarray(x, dtype=np.float32)
    Wq = np.asarray(Wq, dtype=np.float32)
    Wk = np.asarray(Wk, dtype=np.float32)
    Wv = np.asarray(Wv, dtype=np.float32)
    Wo = np.asarray(Wo, dtype=np.float32)
    bo = np.asarray(bo, dtype=np.float32)

    if "nc" not in _CACHE:
        _CACHE["nc"] = build_nc()
    nc = _CACHE["nc"]

    in_maps = _host_prep(x, Wq, Wk, Wv, Wo, policy_mask, memory_weights)
    res = run_bass_kernel_spmd(nc, in_maps, core_ids=list(range(NCORES)))

    acc = np.zeros((B, S, D), dtype=np.float64)
    for c in range(NCORES):
        acc += res.results[c]["y"].astype(np.float64)
    return (acc + bo.astype(np.float64)).astype(np.float32)


# revision 55
# speedup vs baseline: 1.8300x; 1.0246x over previous
"""EnhancedGovernanceAttention Trainium2 kernel (8 NeuronCores, SPMD).

Sharding: core c owns heads {2c, 2c+1} for BOTH batches. Each core computes
its heads' attention and a row-parallel partial of the Wo projection; the
host sums the 8 partials and adds bo.

Math notes (vs the jax reference):
 - softmax max-subtraction is dropped: scores ~ N(0,1) + bias in [0,0.3],
   so exp() cannot overflow; softmax is shift-invariant.
 - log1p memory bias: log(1 + GS*mw + 1e-8) = log(w) is folded into the
   per-(batch,head) additive bias table, so exp(score+bias) already carries
   w for both the PV numerator and the denominator row-sum.
 - causal mask: only lower-triangle k-tiles are computed; the intra-tile
   diagonal mask is baked into the (fp8) bias as -40.
 - scores are computed TRANSPOSED ([k, q]) so the PV matmul directly
   yields attn^T, which is the lhsT the output projection needs.
 - QKV projections run as fp8 DoubleRow matmuls on a hi/lo split of x and
   64*W (3 cross terms; the 64x pre-scale keeps the lo residuals out of
   fp8's subnormal range; 1/64 is folded into the RoPE tables and the V
   staging copy). Everything else runs bf16.
 - the policy bias is added into the score PSUM with an fp8 DoubleRow
   identity matmul (2 k-tiles per slab, [I;0]/[0;I] selects the slot).
 - softmax denominator: exp tiles are accumulated into U (alternating
   DVE/Pool adds); one ones^T @ U matmul per q-block yields the row sums.
"""

import numpy as np
import ml_dtypes
from contextlib import ExitStack

import concourse.bass as bass
import concourse.tile as tile
from concourse import bacc, mybir
from concourse.bass_utils import run_bass_kernel_spmd
from concourse.masks import make_identity

B, S, D, H, HD = 2, 2048, 2048, 16, 128
GS = 0.1
ROPE_BASE = 10000.0
NCORES = 8
HPC = H // NCORES          # heads per core = 2
SCALE = float(HD) ** -0.5
DT = D // 128              # 16 d-tiles
GT = DT // 2               # 8 d-tile pairs (DoubleRow)
ST = S // 128              # 16 s-tiles (also k-tiles)
QB = 512                   # q-block width (phase B)
NQB = S // QB              # 4 q-blocks
SB = 512                   # s-block width (phase A panels)
NSB = S // SB              # 4 s-blocks
MASK_NEG = -40.0
WSC = 64.0                 # fp8 weight pre-scale

F32 = mybir.dt.float32
F32R = mybir.dt.float32r
BF16 = mybir.dt.bfloat16
FP8 = mybir.dt.float8e4
EXP = mybir.ActivationFunctionType.Exp
CPY = mybir.ActivationFunctionType.Copy
DR = mybir.MatmulPerfMode.DoubleRow

_CACHE = {}


def build_nc():
    nc = bacc.Bacc("TRN2", target_bir_lowering=False, debug=False,
                   num_devices=NCORES)

    d_xhi = nc.dram_tensor("xhi", [B, 128, GT, 2, S], FP8, kind="ExternalInput").ap()
    d_xlo = nc.dram_tensor("xlo", [B, 128, GT, 2, S], FP8, kind="ExternalInput").ap()
    CC = HPC * HD
    d_w = {}
    for nm in ("qhi", "qlo", "khi", "klo", "vhi", "vlo"):
        d_w[nm] = nc.dram_tensor(f"w{nm}", [128, GT, 2, CC], FP8,
                                 kind="ExternalInput").ap()
    d_wo = nc.dram_tensor("wo", [128, HPC, D], BF16, kind="ExternalInput").ap()
    d_id8 = nc.dram_tensor("id8", [128, 3, 128], FP8, kind="ExternalInput").ap()
    d_ones = nc.dram_tensor("ones", [128, 1], BF16, kind="ExternalInput").ap()
    d_onesrow = nc.dram_tensor("onesrow", [1, 128], F32R, kind="ExternalInput").ap()
    d_bias = nc.dram_tensor("biasT", [B, HPC, S, S], FP8, kind="ExternalInput").ap()
    d_cs = nc.dram_tensor("cs", [128, 2, S], BF16, kind="ExternalInput").ap()
    d_y = nc.dram_tensor("y", [B, S, D], BF16, kind="ExternalOutput").ap()

    with tile.TileContext(nc) as tc, ExitStack() as ctx:
        consts = ctx.enter_context(tc.tile_pool(name="consts", bufs=1))
        wpool = ctx.enter_context(tc.tile_pool(name="wpool", bufs=1))
        qkv = ctx.enter_context(tc.tile_pool(name="qkv", bufs=2))
        panels = ctx.enter_context(tc.tile_pool(name="panels", bufs=3))
        rope = ctx.enter_context(tc.tile_pool(name="rope", bufs=6))
        svtp = ctx.enter_context(tc.tile_pool(name="svtp", bufs=2))
        slabs = ctx.enter_context(tc.tile_pool(name="slabs", bufs=4))
        expp = ctx.enter_context(tc.tile_pool(name="expp", bufs=4))
        upool = ctx.enter_context(tc.tile_pool(name="upool", bufs=3))
        normp = ctx.enter_context(tc.tile_pool(name="normp", bufs=4))
        outp = ctx.enter_context(tc.tile_pool(name="outp", bufs=10))
        psum_mm = ctx.enter_context(tc.tile_pool(name="psum_mm", bufs=3, space="PSUM"))
        psum_pv = ctx.enter_context(tc.tile_pool(name="psum_pv", bufs=2, space="PSUM"))
        psum_c = ctx.enter_context(tc.tile_pool(name="psum_c", bufs=2, space="PSUM"))
        psum_l = ctx.enter_context(tc.tile_pool(name="psum_l", bufs=1, space="PSUM"))

        # ---------------- constants (emission order = queue priority) ----------
        # ACT queue: q/k weights (first two chains); Pool: cs then v weights.
        t_w = {}
        for nm in ("qhi", "khi", "qlo", "klo", "vhi", "vlo"):
            t_w[nm] = wpool.tile([128, GT, 2, CC], FP8, tag=f"w{nm}", name=f"w{nm}")
            nc.scalar.dma_start(t_w[nm], d_w[nm])
        t_cs = consts.tile([128, 2, S], BF16, tag="cs")   # DMA'd on SP below

        ident_bf = consts.tile([128, 128], BF16, tag="ident_bf")
        make_identity(nc, ident_bf)
        # PE warm-up: ~6us of dep-free junk matmuls so the p-state ramp hits
        # peak clock before the first real chain (and the DMA-bound startup
        # window never resets it).
        warm = psum_l.tile([128, 128], F32, tag="l", name="warm")
        for _ in range(50):
            nc.tensor.matmul(warm, ident_bf, ident_bf, start=True, stop=True,
                             skip_group_check=True)
        # [I, 0, I] in fp8: id8[:, 0:2] = [I;0] (even k-tile), id8[:, 1:3] = [0;I]
        # host-loaded: on-device fp8/f32r init breaks the walrus backend.
        # (DMAs deferred past the startup window: needed only in phase B.)
        id8 = consts.tile([128, 3, 128], FP8, tag="id8")
        onesR = consts.tile([128, 1], BF16, tag="ones")
        onesrow = consts.tile([1, 128], F32R, tag="onesrow")
        t_wo = consts.tile([128, HPC, D], BF16, tag="wo")

        # ---------------- helpers ------------------------------------------
        panel_cache = {}

        def emit_panel(b, sb_i, split=False):
            blk = slice(sb_i * SB, sb_i * SB + SB)
            phi = panels.tile([128, GT, 2, SB], FP8, tag="phi", name="phi")
            plo = panels.tile([128, GT, 2, SB], FP8, tag="plo", name="plo")
            if split:  # halve the first transfers so chains start early
                nc.sync.dma_start(phi[:, 0:GT // 2], d_xhi[b, :, 0:GT // 2, :, blk])
                nc.sync.dma_start(phi[:, GT // 2:], d_xhi[b, :, GT // 2:, :, blk])
                nc.sync.dma_start(plo[:, 0:GT // 2], d_xlo[b, :, 0:GT // 2, :, blk])
                nc.sync.dma_start(plo[:, GT // 2:], d_xlo[b, :, GT // 2:, :, blk])
            else:
                nc.sync.dma_start(phi, d_xhi[b, :, :, :, blk])
                nc.sync.dma_start(plo, d_xlo[b, :, :, :, blk])
            if b == 0 and sb_i < NSB:
                # cs chunk for this block, after the panel: RoPE needs it
                # later than the matmul chains need the panel.
                nc.sync.dma_start(t_cs[:, :, blk], d_cs[:, :, blk])
            return phi, plo

        def dr_part(ps, terms, start, stop, gr=None):
            gr = gr if gr is not None else range(GT)
            n = len(terms) * len(gr)
            idx = 0
            for wt, xt, hc in terms:
                for g in gr:
                    nc.tensor.matmul(
                        ps, wt[:, g, :, hc], xt[:, g, :, :],
                        start=(start and idx == 0),
                        stop=(stop and idx == n - 1),
                        perf_mode=DR, skip_group_check=True)
                    idx += 1

        def dr_chain(ps, whi, wlo, phi, plo, hc):
            # sum of 3 fp8 DoubleRow cross terms; hi*hi first so the chain
            # can start before the lo tensors arrive.
            dr_part(ps, [(whi, phi, hc), (whi, plo, hc), (wlo, phi, hc)],
                    True, True)

        ncp = [0]
        pending_y = []

        def flush_y(k=1):
            # y-DMAs are emitted one C-unit late so the (in-order) issuing
            # SEQ never parks on a not-yet-copied ob tile.
            for _ in range(k):
                if pending_y:
                    dst, ob = pending_y.pop(0)
                    if draining[0]:
                        eng = (nc.gpsimd, nc.sync, nc.scalar)[ncp[0] % 3]
                    else:
                        eng = nc.gpsimd if ncp[0] % 2 == 0 else nc.sync
                    eng.dma_start(dst, ob)

        draining = [False]

        def emit_c_unit(attnT_ref, b_ref, st, nb):
            ss = slice(st * 128, (st + 1) * 128)
            ns = slice(nb * 512, (nb + 1) * 512)
            # during the final drain the score pool is idle: borrow its banks
            # to deepen the C-unit pipeline.
            if draining[0] and ncp[0] % 4 == 0:
                ops = psum_mm.tile([128, 512], F32, tag="mm", name="ops")
            elif draining[0] and ncp[0] % 4 == 1:
                ops = psum_pv.tile([128, 512], F32, tag="pv", name="ops")
            elif draining[0] and ncp[0] % 4 == 2:
                ops = psum_l.tile([128, 512], F32, tag="l", name="ops")
            else:
                ops = psum_c.tile([128, 512], F32, tag="c", name="ops")
            for h in range(HPC):
                nc.tensor.matmul(
                    ops, attnT_ref[h][:, ss], t_wo[:, h, ns],
                    start=(h == 0), stop=(h == HPC - 1))
            ob = outp.tile([128, 512], BF16, tag="ob")
            ncp[0] += 1
            if ncp[0] % 2 == 0:
                nc.scalar.copy(ob, ops)
            else:
                nc.vector.tensor_copy(ob, ops)
            pending_y.append((d_y[b_ref, ss, ns], ob))
            flush_y(1)

        slab_cache = {}

        def emit_slab(b, h, g, qs):
            slab = slabs.tile([128, 2, QB], FP8, tag="slab")
            k0 = g * 256
            nc.sync.dma_start(
                slab,
                d_bias[b, h, k0:k0 + 256, qs].rearrange(
                    "(m p) q -> p m q", p=128))
            return slab

        # deferred work queue: (kind, closure) entries — normalization tails
        # and C-units — that fill PE gaps in later m-loops / phase-A blocks.
        # A tail's lps matmul waits on the previous block's exp/U chain, so
        # tails are only popped when `late` (the consumer has caught up);
        # C-units never jump ahead of their own block's tails.
        fill_q = []

        def pop_fill(k=1, late=True):
            n = 0
            for _ in range(k):
                if not fill_q:
                    return n
                idx = 0
                if late:
                    # promote a nearby tail: emitting its normalization early
                    # unblocks the C-units queued behind it
                    for i in range(min(len(fill_q), 6)):
                        if fill_q[i][0] == "tail":
                            idx = i
                            break
                elif fill_q[0][0] == "tail":
                    return n
                fill_q.pop(idx)[1]()
                n += 1
            return n

        for b in range(B):
            # ============ phase A: x^T panels -> q^T,k^T (RoPE), v ============
            qT = {}
            kT = {}
            vv = {}
            for h in range(HPC):
                qT[h] = qkv.tile([128, S], BF16, tag=f"qT{h}", name=f"qT{h}")
                kT[h] = qkv.tile([128, S], BF16, tag=f"kT{h}", name=f"kT{h}")
                vv[h] = qkv.tile([128, ST, HD], BF16, tag=f"v{h}", name=f"v{h}")

            for sb_i in range(NSB):
                s0 = sb_i * SB
                blk = slice(s0, s0 + SB)
                if (b, sb_i) in panel_cache:
                    phi, plo = panel_cache.pop((b, sb_i))
                else:
                    phi, plo = emit_panel(b, sb_i, split=(b == 0 and sb_i == 0))

                def rope_emit(ps, dest):
                    # cs slot 0 = [cosT;cosT]/64, slot 1 = [-sinT;+sinT]/64
                    t1 = rope.tile([128, SB], F32, tag="t1")
                    t2 = rope.tile([128, SB], F32, tag="t2")
                    nc.vector.tensor_mul(t1, ps, t_cs[:, 0, blk])
                    nc.vector.tensor_mul(
                        t2[0:64, :], ps[64:128, :], t_cs[0:64, 1, blk])
                    nc.vector.tensor_mul(
                        t2[64:128, :], ps[0:64, :], t_cs[64:128, 1, blk])
                    nc.vector.tensor_add(dest[:, blk], t1, t2)

                # q,k chains (both heads) first, then v: the first v chain
                # then starts after wv has streamed in.
                first = b == 0 and sb_i == 0
                if first:
                    # cold start: hi*hi parts of all four q/k chains first
                    # (split by panel half), so the PE works while the lo
                    # tensors are still streaming in.
                    chains = [(pre, h) for h in range(HPC) for pre in ("q", "k")]
                    pss = {}
                    for i, (pre, h) in enumerate(chains):
                        hc = slice(h * HD, (h + 1) * HD)
                        pool, tag = ((psum_mm, "mm") if i < 3 else
                                     (psum_c, "c"))
                        ps = pool.tile([128, SB], F32, tag=tag, name="pss")
                        pss[(pre, h)] = ps
                        dr_part(ps, [(t_w[pre + "hi"], phi, hc)], True, False,
                                gr=range(GT // 2))
                    for pre, h in chains:
                        hc = slice(h * HD, (h + 1) * HD)
                        dr_part(pss[(pre, h)], [(t_w[pre + "hi"], phi, hc)],
                                False, False, gr=range(GT // 2, GT))
                    for pre, h in chains:
                        hc = slice(h * HD, (h + 1) * HD)
                        ps = pss[(pre, h)]
                        dr_part(ps, [(t_w[pre + "hi"], plo, hc),
                                     (t_w[pre + "lo"], phi, hc)], False, True)
                        rope_emit(ps, qT[h] if pre == "q" else kT[h])
                else:
                    for h in range(HPC):
                        hc = slice(h * HD, (h + 1) * HD)
                        for pre, dest in (("q", qT[h]), ("k", kT[h])):
                            ps = psum_mm.tile([128, SB], F32, tag="mm")
                            dr_chain(ps, t_w[pre + "hi"], t_w[pre + "lo"],
                                     phi, plo, hc)
                            rope_emit(ps, dest)
                        pop_fill(1)
                # v directly in natural [s, hd] layout: the panel is the
                # stationary (DoubleRow) operand, wv the moving one.
                for h in range(HPC):
                    hc = slice(h * HD, (h + 1) * HD)
                    vps = psum_mm.tile([128, SB // 128, HD], F32, tag="mm",
                                       name="vps")
                    for c4 in range(SB // 128):
                        cs4 = slice(c4 * 128, (c4 + 1) * 128)
                        idx = 0
                        for wt, xt in ((t_w["vhi"], phi), (t_w["vhi"], plo),
                                       (t_w["vlo"], phi)):
                            for g in range(GT):
                                nc.tensor.matmul(
                                    vps[:, c4, :], xt[:, g, :, cs4],
                                    wt[:, g, :, hc],
                                    start=(idx == 0), stop=(idx == 3 * GT - 1),
                                    perf_mode=DR, skip_group_check=True)
                                idx += 1
                    # last block's copies off ACT so the first exp of
                    # phase B isn't queued behind them
                    dstv = vv[h][:, sb_i * (SB // 128):(sb_i + 1) * (SB // 128), :]
                    if sb_i == NSB - 1:
                        nc.vector.tensor_scalar_mul(dstv, vps, 1.0 / WSC)
                    else:
                        nc.scalar.activation(dstv, vps, CPY, 0.0, 1.0 / WSC)
                    pop_fill(2)
                if b == 0 and sb_i == 0:
                    # on ACT, not Pool: any DMA in Pool's stream lands before
                    # make_identity in the scheduled order and the warm-up
                    # fillers' semaphore would wait on its completion.
                    nc.scalar.dma_start(t_wo, d_wo)   # needed from j=1 on
                    nc.scalar.dma_start(id8, d_id8)
                    nc.scalar.dma_start(onesR, d_ones)
                    nc.scalar.dma_start(onesrow, d_onesrow)
                if sb_i == 2:
                    # prefetch the first two bias slabs of this batch's j=0
                    for g in range(2):
                        slab_cache[(b, 0, 0, g)] = emit_slab(
                            b, 0, g, slice(0, QB))

            # ====== phases B+C software-pipelined over q-blocks ======
            attnT = qT  # norm(j,h) overwrites qT[h][:, qs] after its last read
            for j in range(NQB):
                qs = slice(j * QB, (j + 1) * QB)
                nk = 4 * (j + 1)          # causal: k-tiles 0..nk-1
                for h in range(HPC):
                    pv = psum_pv.tile([128, QB], F32, tag="pv", name="pv")
                    U = upool.tile([128, QB], BF16, tag="U", name="U")
                    offs = [max(0, (m - 4 * j) * 128) for m in range(nk)]

                    def emit_pvu(m, ex):
                        off = offs[m]
                        nc.tensor.matmul(
                            pv[:, off:], vv[h][:, m, :], ex[:, off:],
                            start=(m == 0), stop=(m == nk - 1),
                            skip_group_check=True)
                        if m == 0:
                            nc.vector.tensor_copy(U, ex)
                        else:
                            nc.vector.tensor_add(
                                U[:, off:], U[:, off:], ex[:, off:])

                    # software-skewed m loop: scores/bias/exp of m, then the
                    # PV+U of m-1 (covering the exp latency with C-unit pops).
                    # When no deferred work exists, a junk matmul into the
                    # next score tile (fully overwritten by its start=True)
                    # keeps the PE busy and the p-state ramp at peak.
                    slab = None
                    prev = None
                    sc_next = None
                    if not fill_q:
                        # empty deferred queue: nothing will cover the first
                        # exp latencies — keep the PE warm with junk matmuls
                        # into the first score tile (overwritten by start=True)
                        sc_next = psum_mm.tile([128, QB], F32, tag="mm",
                                               name="sc_next")
                        for _ in range(3 if j == 0 and h == 0 else 2):
                            nc.tensor.matmul(
                                sc_next, ident_bf, qT[h][:, 0:QB],
                                start=True, stop=True, skip_group_check=True)
                    for m in range(nk):
                        if m % 2 == 0:
                            slab = slab_cache.pop((b, j, h, m // 2), None)
                            if slab is None:
                                slab = emit_slab(b, h, m // 2, qs)
                        ml = m % 2
                        off = offs[m]
                        qso = slice(j * QB + off, (j + 1) * QB)
                        if sc_next is not None:
                            sc = sc_next
                            sc_next = None
                        else:
                            sc = psum_mm.tile([128, QB], F32, tag="mm")
                        nc.tensor.matmul(
                            sc[:, off:], kT[h][:, m * 128:(m + 1) * 128],
                            qT[h][:, qso],
                            start=True, stop=False)
                        nc.tensor.matmul(
                            sc[:, off:], id8[:, ml:ml + 2, :],
                            slab[:, :, off:],
                            start=False, stop=True, perf_mode=DR,
                            skip_group_check=True)
                        ex = expp.tile([128, QB], BF16, tag="ex")
                        nc.scalar.activation(ex[:, off:], sc[:, off:], EXP)
                        if prev is not None:
                            if b == B - 1 and j == NQB - 1 and h == HPC - 1:
                                # keep a few C-units back: they fill the PE
                                # during the final tail's serial chain
                                k = 1 if m % 2 else 0
                            else:
                                k = 2 if len(fill_q) > 8 else 1
                            pop_fill(k, late=(m >= 3))
                            emit_pvu(*prev)
                        prev = (m, ex)
                    pop_fill(2, late=False)
                    emit_pvu(*prev)

                    last = (b == B - 1 and j == NQB - 1 and h == HPC - 1)

                    def make_tail(pv=pv, U=U, h=h, qs=qs, at=attnT, last=last):
                        def tail():
                            lrow = psum_l.tile([1, QB], F32, tag="l", name="l")
                            nc.tensor.matmul(lrow, onesR, U)
                            rl = normp.tile([1, QB], F32, tag="rl")
                            rb = normp.tile([128, QB], F32, tag="rb")
                            if last:
                                # end-critical: chunk the chain so the first
                                # drain C-units (one 128-col attnT slice each)
                                # start before the later chunks normalize
                                for c in range(QB // 128):
                                    cs4 = slice(c * 128, (c + 1) * 128)
                                    qc = slice(qs.start + c * 128,
                                               qs.start + (c + 1) * 128)
                                    nc.vector.reciprocal(rl[:, cs4],
                                                         lrow[:, cs4])
                                    nc.gpsimd.partition_broadcast(
                                        rb[:, cs4], rl[:, cs4])
                                    nc.vector.tensor_mul(
                                        at[h][:, qc], pv[:, cs4], rb[:, cs4])
                            else:
                                nc.vector.reciprocal(rl, lrow)
                                nc.gpsimd.partition_broadcast(rb, rl)
                                nc.vector.tensor_mul(at[h][:, qs], pv, rb)
                        return tail

                    fill_q.append(("tail", make_tail()))

                    # prefetch next batch's first panels near the end
                    if b + 1 < B and j == NQB - 1 and h == 0:
                        panel_cache[(b + 1, 0)] = emit_panel(b + 1, 0)

                fill_q.extend(
                    ("c", lambda st=st, nb=nb, at=attnT, bb=b:
                     emit_c_unit(at, bb, st, nb))
                    for st in range(4 * j, 4 * j + 4) for nb in range(D // 512))

        draining[0] = True
        while fill_q:
            fill_q.pop(0)[1]()
        flush_y(len(pending_y))

    nc.compile()
    return nc


def _host_prep(x, Wq, Wk, Wv, Wo, policy_mask, memory_weights):
    """Build the per-core input maps."""
    bf = ml_dtypes.bfloat16
    f8 = ml_dtypes.float8_e4m3

    def hilo_tiles(a):
        # [D, C] (or [D, S]) -> hi/lo fp8 in [128, GT, 2, C] DoubleRow layout
        hi = a.astype(f8)
        lo = (a - hi.astype(np.float32)).astype(f8)
        def tl(t):
            return np.ascontiguousarray(
                t.reshape(GT, 2, 128, -1).transpose(2, 0, 1, 3))
        return tl(hi), tl(lo)

    xhi = np.empty((B, 128, GT, 2, S), f8)
    xlo = np.empty((B, 128, GT, 2, S), f8)
    for b in range(B):
        xt = np.ascontiguousarray(np.asarray(x[b], np.float32).T)  # [D, S]
        xhi[b], xlo[b] = hilo_tiles(xt)

    # RoPE tables (carry the 1/WSC weight descale):
    inv_freq = (1.0 / (ROPE_BASE ** (np.arange(0, HD, 2, dtype=np.float32) / HD)))
    t = np.arange(S, dtype=np.float32)
    freqs = np.outer(t, inv_freq).astype(np.float32)      # [S, 64]
    cosT = np.cos(freqs).T.astype(np.float32) / WSC       # [64, S]
    sinT = np.sin(freqs).T.astype(np.float32) / WSC
    cs = np.empty((128, 2, S), np.float32)
    cs[0:64, 0] = cosT
    cs[64:128, 0] = cosT
    cs[0:64, 1] = -sinT
    cs[64:128, 1] = sinT
    cs = cs.astype(bf)

    # memory multiplier w = 1 + GS*mw + 1e-8  (exp(log1p(z)) = 1+z)
    mw = memory_weights.reshape(B, S).astype(np.float64)
    logw = np.log(1.0 + GS * mw + 1e-8).astype(np.float32)  # [B, S]

    # transposed, causal-masked, pre-scaled policy bias per (batch, head)
    maskT = np.tril(np.full((S, S), MASK_NEG, dtype=np.float32), -1)
    pol = np.asarray(policy_mask, dtype=np.float32)[0]    # [H, S, S]

    id8h = np.zeros((128, 3, 128), np.float32)
    id8h[:, 0, :] = np.eye(128, dtype=np.float32)
    id8h[:, 2, :] = np.eye(128, dtype=np.float32)
    id8h = id8h.astype(f8)
    ones1 = np.ones((128, 1), np.float32)

    in_maps = []
    for c in range(NCORES):
        cols = slice(c * HPC * HD, (c + 1) * HPC * HD)
        bias_c = np.empty((B, HPC, S, S), dtype=f8)
        for hl in range(HPC):
            hg = c * HPC + hl
            polT = GS * pol[hg].T + maskT                 # [S(k), S(q)]
            for b in range(B):
                bias_c[b, hl] = (polT + logw[b][:, None]).astype(f8)
        wo_c = np.ascontiguousarray(
            np.asarray(Wo, np.float32)[cols, :]
            .reshape(HPC, 128, D).transpose(1, 0, 2)).astype(bf)
        m = {"xhi": xhi, "xlo": xlo, "wo": wo_c, "biasT": bias_c, "cs": cs,
             "id8": id8h, "ones": ones1.astype(bf),
             "onesrow": ones1.reshape(1, 128)}
        for nm, w, s in (("q", Wq, WSC), ("k", Wk, WSC * SCALE), ("v", Wv, WSC)):
            hi, lo = hilo_tiles(np.asarray(w, np.float32)[:, cols] * np.float32(s))
            m[f"w{nm}hi"] = hi
            m[f"w{nm}lo"] = lo
        in_maps.append(m)
    return in_maps


def kernel(x, Wq, Wk, Wv, Wo, bo, policy_mask, memory_weights):
    x = np.asarray(x, dtype=np.float32)
    Wq = np.asarray(Wq, dtype=np.float32)
    Wk = np.asarray(Wk, dtype=np.float32)
    Wv = np.asarray(Wv, dtype=np.float32)
    Wo = np.asarray(Wo, dtype=np.float32)
    bo = np.asarray(bo, dtype=np.float32)

    if "nc" not in _CACHE:
        _CACHE["nc"] = build_nc()
    nc = _CACHE["nc"]

    in_maps = _host_prep(x, Wq, Wk, Wv, Wo, policy_mask, memory_weights)
    res = run_bass_kernel_spmd(nc, in_maps, core_ids=list(range(NCORES)))

    acc = np.zeros((B, S, D), dtype=np.float64)
    for c in range(NCORES):
        acc += res.results[c]["y"].astype(np.float64)
    return (acc + bo.astype(np.float64)).astype(np.float32)
